# revision 1
# baseline (speedup 1.0000x reference)
"""Trainium2 Bass kernel: GQA causal attention (B=2, S=2048, H=2048, 16 q-heads,
4 kv-heads, head_dim=128), tensor-parallel over 8 NeuronCores.

Sharding: 2 q-heads + their (shared) kv-head per core; wq/wk/wv column-sharded,
wo row-sharded.  Each core computes a partial o_proj output; the host sums the
8 partials (the standard TP partial-sum unshard).

All matmul operands are bf16 (PSUM accumulation stays fp32): same PE rate as
fp32r but half the DMA bytes, 2-4x DVE throughput on elementwise ops, and full
PE rate at any moving width (so causal tiles narrow to 128).

On-chip layouts are transposed (feature-on-partition) except V:
  q/k:   qkvT = w.T @ x.T            (PE, accumulate over 16 h-chunks)
  v:     natural [s, d] directly     (PE, xT chunks stationary, wv moving;
                                      no PE transposes needed)
  RoPE:  q' = q*cos + rot(q)*sin     (pure DVE: rot(q) via partition-offset
                                      muls against a sign-folded sin table)
  scoresT[k,q] = K @ Q^T             (PE; wq pre-scaled by 1/sqrt(D))
  P^T   = exp(scoresT - 40)          (ACT, fused bias; exact softmax after
                                      normalization: const cancels)
  causal mask: affine_select on P^T  (Pool/GpSimd, fill=0)
  outT  = V^T @ P^T                  (PE, PSUM-accumulated over k-chunks)
  rowsum: quad-packed ones-matmuls   (DVE pre-sums quads of P^T tiles so the
                                      PE streams 1/4 of the columns)
  outT *= bcast(1/rowsum)            (DVE recip/mul)
  out_partial = outT.T @ wo_c        (PE; outT is already the needed lhsT)

DMAs are batched (whole x row-block / whole output row) to amortize the
~625ns-per-DMA HWDGE cost; o_proj psum->sbuf copies rotate across DVE, Pool
and ACT so no single engine becomes co-critical with the PE.
"""

import os
import sys
import time

import numpy as np

sys.path.insert(0, "/opt/trn_rl_repo")

from contextlib import ExitStack

import concourse.bass as bass
from concourse import bacc
import concourse.mybir as mybir
import concourse.tile as tile
from concourse.bass_utils import run_bass_kernel_spmd

F32 = mybir.dt.float32
BF16 = mybir.dt.bfloat16
AF = mybir.ActivationFunctionType
ALU = mybir.AluOpType

B, S, H = 2, 2048, 2048
NH, KVH, D = 16, 4, 128
NCORES = 8
HPC = NH // NCORES  # q heads per core = 2
R = B * S  # 4096 flattened rows
QKV_W = HPC * D + 2 * D  # 512 = [q0|q1|k|v] columns per core
NB_RB = R // 512  # 8 row-blocks of 512
NB_HC = H // 128  # 16 contraction chunks
SB = S // 512  # 4 q-blocks per batch
SC = S // 128  # 16 k-chunks per batch
EXP_BIAS = -40.0

LAST_EXEC_TIME_NS = None
LAST_RESULTS = None


def build_graph(reps=1):
    nc = bacc.Bacc(
        "TRN2", target_bir_lowering=False, debug=False, num_devices=NCORES
    )
    # host-prepared layouts (see kernel()): xTr[rb*128+p, hc*512+c] =
    # x.T[hc*128+p, rb*512+c]; wqkvr[p, hc*512+c] = wqkv[hc*128+p, c];
    # wor[p, h*2048+c] = wo[h*128+p, c].
    xTr = nc.dram_tensor("xTr", [NB_RB * 128, NB_HC * 512], BF16, kind="ExternalInput").ap()
    # kT-dedup: each core's xTr is permuted so its own batch comes first
    # (even cores: batch 0, odd: batch 1 — the pair shares one kv head);
    # each core projects+ropes kT only for that local batch, the pair
    # AllGathers the halves, and the partner half is reconstructed exactly
    # as (slot0+slot1)-local in fp32. The host unpermutes the output rows.
    kvloc = nc.dram_tensor("kvloc", [128, S], BF16, kind="Internal").ap()
    kvglob = nc.dram_tensor("kvglob", [256, S], BF16, kind="Internal").ap()
    vred = nc.dram_tensor("vred", [128, S], BF16, kind="Internal").ap()
    vsum = nc.dram_tensor("vsum", [128, S], BF16, kind="Internal").ap()
    wqkvr = nc.dram_tensor("wqkvr", [128, NB_HC * 512], BF16, kind="ExternalInput").ap()
    wor = nc.dram_tensor("wor", [128, HPC * H], BF16, kind="ExternalInput").ap()
    cosT = nc.dram_tensor("cosT", [D, S], BF16, kind="ExternalInput").ap()
    sinadjT = nc.dram_tensor("sinadjT", [D, S], BF16, kind="ExternalInput").ap()
    out = nc.dram_tensor("out", [R, H], BF16, kind="ExternalOutput").ap()

    with tile.TileContext(nc) as tc, ExitStack() as ctx:
        # ---- persistent SBUF ----
        const_pool = ctx.enter_context(tc.tile_pool(name="const", bufs=1))
        w_sb = const_pool.tile([128, NB_HC * 512], BF16)
        wo_sb = const_pool.tile([128, HPC * H], BF16)
        cos_sb = const_pool.tile([128, S], BF16)
        sinadj_sb = const_pool.tile([128, S], BF16)
        ones_sb = const_pool.tile([128, 128], BF16)  # rowsum lhsT / bcast
        expb_sb = const_pool.tile([128, 1], F32)  # exp bias (per-partition)
        # qk/v live in per-row-block tiles: tile-granular dependency tracking
        # would otherwise serialize attention's first reads behind the LAST
        # row-block's RoPE/copy on the DVE queue.
        qk_sb = {
            (cg, rb): const_pool.tile([128, 512], BF16, name=f"qk{cg}_{rb}")
            for cg in range(2)
            for rb in range(NB_RB)
        }
        kT_sb = const_pool.tile([128, R], BF16)  # gathered roped kT, both batches
        vall_sb = const_pool.tile([128, R], BF16)  # gathered natural V
        g0_sb = const_pool.tile([128, S], BF16)
        g1_sb = const_pool.tile([128, S], BF16)
        scr_sb = const_pool.tile([128, 1], F32)

        nc.gpsimd.memset(ones_sb[:], 1.0)
        nc.gpsimd.memset(expb_sb[:], EXP_BIAS)
        # touch Exp once so the ACT table load happens while ACT is idle,
        # not in front of the first real softmax tile
        nc.scalar.activation(scr_sb[:], expb_sb[:], AF.Exp, bias=0.0, scale=1.0)

        outT_pool = ctx.enter_context(tc.tile_pool(name="outT", bufs=2))
        ms_ps_pool = ctx.enter_context(tc.tile_pool(name="ms_ps", bufs=2, space="PSUM"))
        osb_pool = ctx.enter_context(tc.tile_pool(name="osb", bufs=6))
        xt_pool = ctx.enter_context(tc.tile_pool(name="xt", bufs=2))
        # scratch SBUF pools are persistent: per-phase pools would reuse the
        # same addresses and stall each phase's first ops on the previous
        # phase's last frees
        rtmp_pool = ctx.enter_context(tc.tile_pool(name="rtmp", bufs=8))
        pt_pool = ctx.enter_context(tc.tile_pool(name="pt", bufs=14))
        s2_pool = ctx.enter_context(tc.tile_pool(name="s2", bufs=6))
        s4_pool = ctx.enter_context(tc.tile_pool(name="s4", bufs=8))
        rr_sb_pool = ctx.enter_context(tc.tile_pool(name="rr_sb", bufs=2))

        # ---- o_proj drip FIFO: one (row-block, nb) pair per emission so the
        # in-order PE queue always has other matmuls between an o_proj pair
        # and its psum-slot dependency (the psum->sbuf copy). Output rows are
        # staged in a [128, 2048] row buffer and DMA'd once per row-block.
        pending = []
        ncopy = [0]
        outT_by_b = {}

        def emit_op(nmax, split=False, pool=None, defer_below=0):
            for _ in range(nmax):
                if len(pending) <= defer_below:
                    return
                ob, oT, st, nb = pending.pop(0)
                op_ps = (pool or ms_ps_pool).tile(
                    [128, 512], F32, tag="ms", name="op_ps"
                )
                for h in range(HPC):
                    nc.tensor.matmul(
                        op_ps[:],
                        oT[:, h * S + st * 128 : h * S + (st + 1) * 128],
                        wo_sb[:, h * H + nb * 512 : h * H + (nb + 1) * 512],
                        start=(h == 0),
                        stop=(h == HPC - 1),
                    )
                osb = osb_pool.tile([128, 512], BF16, tag="osb", name="osb")
                if split:  # tail flush: alternate engines per tile
                    if ncopy[0] % 2 == 0:
                        nc.vector.tensor_copy(osb[:], op_ps[:])
                    else:
                        nc.scalar.copy(osb[:], op_ps[:])
                else:
                    # psum drains rotate 2:1 over DVE and ACT (ACT also
                    # carries the softmax exps; Pool cannot read PSUM)
                    if ncopy[0] % 3 == 1:
                        nc.scalar.copy(osb[:], op_ps[:])
                    else:
                        nc.vector.tensor_copy(osb[:], op_ps[:])
                ncopy[0] += 1
                r0 = ob * S + st * 128
                nc.sync.dma_start(
                    out[r0 : r0 + 128, nb * 512 : (nb + 1) * 512], osb[:]
                )

        xts = {}

        def fetch(src, row0, key, granularity=2):
            t = xt_pool.tile([128, NB_HC * 512], BF16, tag="xt")
            step = NB_HC // granularity
            for g in range(granularity):
                sl = slice(g * step * 512, (g + 1) * step * 512)
                nc.sync.dma_start(t[:, sl], src[row0 : row0 + 128, sl])
            xts[key] = t

        def rope(ps, qraw, dst, pos, out_pool):
            # RoPE: q' = q*cos + rot(q)*sin; rotate-half reads come
            # partition-offset straight from PSUM (the equal-base rule only
            # binds when both inputs are SBUF); psum reads go first so the
            # bank frees as early as possible. sinadj has rotate_half's sign
            # folded in: sinadj[0:64] = -sin[0:64], sinadj[64:128] = +sin.
            cs = cos_sb[:, pos * 512 : (pos + 1) * 512]
            sn_lo = sinadj_sb[0:64, pos * 512 : (pos + 1) * 512]
            sn_hi = sinadj_sb[64:128, pos * 512 : (pos + 1) * 512]
            t1 = rtmp_pool.tile([128, 512], BF16, tag="rtmp")
            t2 = rtmp_pool.tile([128, 512], BF16, tag="rtmp")
            nc.vector.tensor_mul(t2[0:64, :], ps[64:128, :], sn_lo)
            nc.vector.tensor_mul(t2[64:128, :], ps[0:64, :], sn_hi)
            nc.vector.tensor_mul(t1[:], qraw[:], cs)
            nc.vector.tensor_add(dst, t1[:], t2[:])

        for _rep in range(reps):
            for b in range(B):
                # ---- phase 1: q + V (+ local kT for b==0) projections and
                # RoPE; the pair AllGather of roped kT halves runs under the
                # local-batch attention ----
                with (
                    tc.tile_pool(name="q_ps", bufs=5, space="PSUM") as q_ps_pool,
                ):
                    if b == 1:
                        # exchange the roped local-kT halves within the pair;
                        # emitted here so no queue parks on it during the
                        # local-batch attention
                        nc.gpsimd.collective_compute(
                            "AllGather", ALU.bypass,
                            [[2 * p, 2 * p + 1] for p in range(NCORES // 2)],
                            ins=[kvloc], outs=[kvglob],
                        )
                        nc.gpsimd.collective_compute(
                            "AllReduce", ALU.add,
                            [[2 * p, 2 * p + 1] for p in range(NCORES // 2)],
                            ins=[vred], outs=[vsum],
                        )
                    for rbl in range(SB):
                        rb = b * SB + rbl
                        if b == 1 and rbl == 3:
                            # partner V = AllReduce(sum) - local, exact in fp32
                            # up to the collective's bf16 sum rounding; gated
                            # like the kT unpack so the scheduler cannot park
                            # the collective-wait mid-attention
                            gate = outT_by_b[0]
                            for tb in range(SB):
                                sl = slice(tb * 512, (tb + 1) * 512)
                                vst = rtmp_pool.tile(
                                    [128, 512], BF16, tag="rtmp", name="vst"
                                )
                                nc.vector.tensor_copy(
                                    vst[0:1, 0:1], gate[0:1, HPC * S - 1 : HPC * S]
                                )
                                nc.scalar.dma_start(vst[:], vsum[:, sl])
                                nc.vector.tensor_sub(
                                    vall_sb[:, S + tb * 512 : S + (tb + 1) * 512],
                                    vst[:],
                                    vall_sb[:, sl],
                                )
                        if b == 1 and rbl == 2:
                            # reconstruct the partner's roped kT exactly:
                            # bf16+bf16 in fp32 is exact, so (g0+g1)-local is
                            # bit-exact the partner half
                            gate = outT_by_b[0]
                            nc.vector.tensor_copy(
                                g0_sb[0:1, 0:1], gate[0:1, HPC * S - 1 : HPC * S]
                            )
                            nc.vector.tensor_copy(
                                g1_sb[0:1, 0:1], gate[0:1, HPC * S - 1 : HPC * S]
                            )
                            nc.scalar.dma_start(g0_sb[:], kvglob[0:128, :])
                            nc.scalar.dma_start(g1_sb[:], kvglob[128:256, :])
                            for tb in range(SB):
                                sl = slice(tb * 512, (tb + 1) * 512)
                                gs = rtmp_pool.tile([128, 512], F32, tag="gsum", name="gs")
                                nc.vector.tensor_add(gs[:], g0_sb[:, sl], g1_sb[:, sl])
                                nc.vector.tensor_sub(
                                    kT_sb[:, S + tb * 512 : S + (tb + 1) * 512],
                                    gs[:],
                                    kT_sb[:, sl],
                                )
                        if rb == 0:
                            # startup: stream w and x at fine granularity so
                            # the first matmuls' deps land early
                            t = xt_pool.tile([128, NB_HC * 512], BF16, tag="xt")
                            xts[("q", 0)] = t
                            for lo, hi in [(0, 1), (1, 2), (2, 4), (4, 6),
                                           (6, 8), (8, 10), (10, 12),
                                           (12, 14), (14, 16)]:
                                sl = slice(lo * 512, hi * 512)
                                nc.sync.dma_start(w_sb[:, sl], wqkvr[:, sl])
                                nc.sync.dma_start(t[:, sl], xTr[0:128, sl])
                        xt = xts.pop(("q", rb))
                        if rb + 1 < NB_RB:
                            fetch(xTr, (rb + 1) * 128, ("q", rb + 1))
                        if rb == 0:
                            nc.sync.dma_start(cos_sb[:], cosT)
                            nc.sync.dma_start(sinadj_sb[:], sinadjT)
                            nc.sync.dma_start(wo_sb[:], wor)
                        q0_ps = q_ps_pool.tile([128, 512], F32, tag="qps", name="q0")
                        q1_ps = q_ps_pool.tile([128, 512], F32, tag="qps", name="q1")
                        if b == 0:
                            v_ps = q_ps_pool.tile([128, 512], F32, tag="qps", name="v")
                        q_list = [q0_ps, q1_ps]
                        if b == 0:
                            k_ps = q_ps_pool.tile([128, 512], F32, tag="qps", name="k")
                            for hc in range(NB_HC):
                                nc.tensor.matmul(
                                    k_ps[:],
                                    w_sb[:, hc * 512 + 256 : hc * 512 + 384],
                                    xt[:, hc * 512 : (hc + 1) * 512],
                                    start=(hc == 0),
                                    stop=(hc == NB_HC - 1),
                                )
                        for hc in range(NB_HC):
                            xsl = xt[:, hc * 512 : (hc + 1) * 512]
                            for cg in range(2):
                                nc.tensor.matmul(
                                    q_list[cg][:],
                                    w_sb[:, hc * 512 + cg * 128 : hc * 512 + (cg + 1) * 128],
                                    xsl,
                                    start=(hc == 0),
                                    stop=(hc == NB_HC - 1),
                                )
                            emit_op(1)
                        # v: local batch only (the partner half arrives via
                        # a pair AllReduce and exact subtraction); one
                        # accumulation group at a time — interleaving
                        # independent start/stop groups in different column
                        # regions of one PSUM bank miscomputes on HW
                        if b == 0:
                            for rc in range(4):
                                for hc in range(NB_HC):
                                    nc.tensor.matmul(
                                        v_ps[:, rc * 128 : (rc + 1) * 128],
                                        xt[:, hc * 512 + rc * 128 : hc * 512 + (rc + 1) * 128],
                                        w_sb[:, hc * 512 + 384 : hc * 512 + 512],
                                        start=(hc == 0),
                                        stop=(hc == NB_HC - 1),
                                    )
                                emit_op(1)
                            nc.scalar.copy(
                                vall_sb[:, rb * 512 : (rb + 1) * 512], v_ps[:]
                            )
                            nc.scalar.dma_start(
                                vred[:, rb * 512 : (rb + 1) * 512],
                                vall_sb[:, rb * 512 : (rb + 1) * 512],
                            )
                        raws = {}
                        if b == 0:
                            kraw = rtmp_pool.tile([128, 512], BF16, tag="rtmp")
                            nc.scalar.copy(kraw[:], k_ps[:])
                            rope(
                                k_ps, kraw,
                                kT_sb[:, rbl * 512 : (rbl + 1) * 512],
                                rbl, rtmp_pool,
                            )
                            nc.scalar.dma_start(
                                kvloc[:, rbl * 512 : (rbl + 1) * 512],
                                kT_sb[:, rbl * 512 : (rbl + 1) * 512],
                            )
                        for cg in range(2):
                            qraw = rtmp_pool.tile([128, 512], BF16, tag="rtmp")
                            nc.scalar.copy(qraw[:], q_list[cg][:])
                            raws[cg] = qraw
                        for cg in range(2):
                            rope(
                                q_list[cg], raws[cg], qk_sb[(cg, rb)][:],
                                rbl, rtmp_pool,
                            )

                # ---- phase 2: attention for batch b ----
                # pool-open order controls bank placement: rs (written last)
                # takes the banks freed last by phase 1; st (needed first)
                # lands on the earliest-freed/spare banks
                with (
                    tc.tile_pool(name="rs_ps", bufs=2, space="PSUM") as rs_ps_pool,
                    tc.tile_pool(name="ot_ps", bufs=2, space="PSUM") as ot_ps_pool,
                    tc.tile_pool(name="st_ps", bufs=2, space="PSUM") as st_ps_pool,
                ):
                    PD = 5  # per-head pipeline depth
                    outT = outT_pool.tile([128, HPC * S], BF16)
                    outT_by_b[b] = outT
                    for qb in range(SB):
                        o_ps, r_ps = {}, {}
                        for h in range(HPC):
                            o_ps[h] = ot_ps_pool.tile(
                                [128, 512], F32, tag="ot", name=f"ot{h}"
                            )
                            r_ps[h] = rs_ps_pool.tile(
                                [128, 512], F32, tag="rs", name=f"rs{h}"
                            )
                        nj = 4 * qb + 4
                        pd = 3 if (b == B - 1 and qb == SB - 1) else PD
                        dfb = 10 if b < B - 1 else 0
                        pts = {}
                        s2s = {}
                        s4s = {}
                        diag = {}
                        for jj in range(nj + pd):
                            emit_op(3 if jj < 2 else 1, defer_below=dfb)
                            if jj < nj:
                                j = jj
                                r = j - 4 * qb  # diagonal band index
                                qoff = 128 * r if r > 0 else 0
                                W = 512 - qoff
                                for h in range(HPC):
                                    s_ps = st_ps_pool.tile([128, 512], F32)
                                    nc.tensor.matmul(
                                        s_ps[:, qoff:512],
                                        kT_sb[:, b * S + j * 128 : b * S + (j + 1) * 128],
                                        qk_sb[(h, b * SB + qb)][:, qoff:512],
                                        start=True,
                                        stop=True,
                                    )
                                    pt = pt_pool.tile([128, 512], BF16)
                                    nc.scalar.activation(
                                        pt[:, qoff:512],
                                        s_ps[:, qoff:512],
                                        AF.Exp,
                                        bias=expb_sb[:],
                                        scale=1.0,
                                    )
                                    if r >= 0:
                                        # zero where k > q inside the 128-wide
                                        # diagonal ramp
                                        nc.gpsimd.affine_select(
                                            out=pt[:, qoff : qoff + 128],
                                            in_=pt[:, qoff : qoff + 128],
                                            pattern=[[1, 128]],
                                            compare_op=ALU.is_ge,
                                            fill=0.0,
                                            base=0,
                                            channel_multiplier=-1,
                                        )
                                    pts[(h, j)] = (pt, qoff, W)
                                    # rowsum packing on DVE (all-bf16 = fast):
                                    padd = nc.vector.tensor_add
                                    pcopy = nc.vector.tensor_copy
                                    if j < 4 * qb:
                                        if j % 2 == 1:
                                            s2 = s2_pool.tile([128, 512], BF16, tag="s2")
                                            padd(s2[:], pts[(h, j - 1)][0][:], pt[:])
                                            s2s[(h, j // 2)] = s2
                                        if j % 4 == 3:
                                            s4 = s4_pool.tile([128, 512], BF16, tag="s4")
                                            padd(
                                                s4[:],
                                                s2s.pop((h, j // 2 - 1))[:],
                                                s2s.pop((h, j // 2))[:],
                                            )
                                            s4s[(h, j // 4)] = s4
                                    elif r == 1:
                                        pt0 = pts[(h, 4 * qb)][0]
                                        sa = s4_pool.tile([128, 512], BF16, tag="s4")
                                        pcopy(sa[:, 0:128], pt0[:, 0:128])
                                        padd(
                                            sa[:, 128:512],
                                            pt0[:, 128:512],
                                            pt[:, 128:512],
                                        )
                                        diag[(h, 0)] = sa
                                    elif r == 3:
                                        pt2 = pts[(h, 4 * qb + 2)][0]
                                        sb_ = s4_pool.tile([128, 512], BF16, tag="s4")
                                        pcopy(sb_[:, 256:384], pt2[:, 256:384])
                                        padd(
                                            sb_[:, 384:512],
                                            pt2[:, 384:512],
                                            pt[:, 384:512],
                                        )
                                        diag[(h, 1)] = sb_
                            if jj >= pd:
                                j2 = jj - pd
                                for h in range(HPC):
                                    pt2, qoff2, W2 = pts.pop((h, j2))
                                    if j2 < 4 * qb:
                                        if j2 % 4 == 3:
                                            s4c = s4s.pop((h, j2 // 4))
                                            nc.tensor.matmul(
                                                r_ps[h][:],
                                                ones_sb[:],
                                                s4c[:],
                                                start=(j2 == 3),
                                                stop=False,
                                                skip_group_check=True,
                                            )
                                    elif j2 == 4 * qb + 1:
                                        nc.tensor.matmul(
                                            r_ps[h][:],
                                            ones_sb[:],
                                            diag[(h, 0)][:],
                                            start=(qb == 0),
                                            stop=False,
                                            skip_group_check=True,
                                        )
                                    elif j2 == 4 * qb + 3:
                                        nc.tensor.matmul(
                                            r_ps[h][:, 256:512],
                                            ones_sb[:],
                                            diag[(h, 1)][:, 256:512],
                                            start=False,
                                            stop=True,
                                            skip_group_check=True,
                                        )
                                    nc.tensor.matmul(
                                        o_ps[h][:, qoff2:512],
                                        vall_sb[:, b * S + j2 * 128 : b * S + (j2 + 1) * 128],
                                        pt2[:, qoff2:512],
                                        start=(j2 == 0),
                                        stop=(j2 == nj - 1),
                                        skip_group_check=True,
                                    )
                            emit_op(1, defer_below=dfb)
                        for h in range(HPC):
                            rr = rr_sb_pool.tile([128, 512], F32, tag="rr")
                            nc.vector.reciprocal(rr[:], r_ps[h][:])
                            nc.vector.tensor_mul(
                                outT[:, h * S + qb * 512 : h * S + (qb + 1) * 512],
                                o_ps[h][:],
                                rr[:],
                            )
                        for stq in range(4):
                            for nb in range(4):
                                pending.append((b, outT, qb * 4 + stq, nb))
            # final drain: attention pools are closed, so spread the o_proj
            # psums over 6 banks to decouple the matmul stream from the
            # copy+DMA latency chain
            with tc.tile_pool(name="drain_ps", bufs=6, space="PSUM") as drain_pool:
                emit_op(len(pending), split=True, pool=drain_pool)
    nc.compile()
    return nc


_GRAPH = None


def _rope_tables():
    inv_freq = 1.0 / (10000.0 ** (np.arange(0, D, 2, dtype=np.float32) / D))
    t = np.arange(S, dtype=np.float32)
    freqs = np.outer(t, inv_freq)
    emb = np.concatenate([freqs, freqs], axis=-1)  # (S, D)
    cosT = np.ascontiguousarray(np.cos(emb).T.astype(np.float32))
    sinT = np.ascontiguousarray(np.sin(emb).T.astype(np.float32))
    sinadjT = sinT.copy()
    sinadjT[0:64, :] *= -1.0  # fold rotate_half's sign into the table
    return cosT, sinadjT


def kernel(x, wq, wk, wv, wo):
    global _GRAPH, LAST_EXEC_TIME_NS, LAST_RESULTS
    import ml_dtypes

    bf16 = ml_dtypes.bfloat16
    x = np.asarray(x, dtype=np.float32)
    wq = np.asarray(wq, dtype=np.float32)
    wk = np.asarray(wk, dtype=np.float32)
    wv = np.asarray(wv, dtype=np.float32)
    wo = np.asarray(wo, dtype=np.float32)

    xT = np.ascontiguousarray(x.reshape(R, H).T)
    # xTr[rb*128+p, hc*512+c] = xT[hc*128+p, rb*512+c]
    xTr = np.ascontiguousarray(
        xT.reshape(NB_HC, 128, NB_RB, 512).transpose(2, 1, 0, 3).reshape(
            NB_RB * 128, NB_HC * 512
        )
    ).astype(bf16)
    cosT, sinadjT = _rope_tables()
    cosT = cosT.astype(bf16)
    sinadjT = sinadjT.astype(bf16)
    scale = np.float32(1.0 / np.sqrt(D))

    in_maps = []
    for c in range(NCORES):
        kv = c // HPC
        wq_c = wq[:, c * HPC * D : (c + 1) * HPC * D] * scale
        wk_c = wk[:, kv * D : (kv + 1) * D]
        wv_c = wv[:, kv * D : (kv + 1) * D]
        wqkv_c = np.concatenate([wq_c, wk_c, wv_c], axis=1, dtype=np.float32)
        # wqkvr[p, hc*512+c] = wqkv_c[hc*128+p, c]
        wqkvr = np.ascontiguousarray(
            wqkv_c.reshape(NB_HC, 128, QKV_W).transpose(1, 0, 2).reshape(
                128, NB_HC * 512
            )
        ).astype(bf16)
        wo_c = wo[c * HPC * D : (c + 1) * HPC * D, :]
        wor = np.ascontiguousarray(
            wo_c.reshape(HPC, 128, H).transpose(1, 0, 2).reshape(128, HPC * H)
        ).astype(bf16)
        # kT dedup: each core sees its OWN batch first (even cores batch 0,
        # odd cores batch 1) and ropes kT only for it; the pair AllGather +
        # exact reconstruction supplies the partner half. The output rows
        # come back local-batch-first and are unpermuted below.
        if c % 2 == 0:
            xTr_c = xTr
        else:
            xTr_c = np.ascontiguousarray(
                np.concatenate([xTr[SB * 128 :], xTr[: SB * 128]], axis=0)
            )
        in_maps.append(
            {
                "xTr": xTr_c,
                "wqkvr": wqkvr,
                "wor": wor,
                "cosT": cosT,
                "sinadjT": sinadjT,
            }
        )

    if _GRAPH is None:
        _GRAPH = build_graph()

    # NTFF tracing is unavailable on axon clients without antenv.axon_hooks;
    # make sure an inherited BASS_TRACE can't break execution.
    os.environ["BASS_NEVER_TRACE"] = "1"
    res = None
    for attempt in range(3):
        try:
            res = run_bass_kernel_spmd(
                _GRAPH, in_maps, core_ids=list(range(NCORES))
            )
            break
        except Exception:
            # transient axon-terminal failures (mesh desync / LoadExecutable)
            # usually clear on retry
            if attempt == 2:
                raise
            time.sleep(5.0)
    LAST_EXEC_TIME_NS = res.exec_time_ns
    LAST_RESULTS = res
    acc = np.zeros((R, H), dtype=np.float32)
    for c in range(NCORES):
        part = np.asarray(res.results[c]["out"], dtype=np.float32)
        if c % 2 == 1:  # odd cores computed batch 1 in their first half
            part = np.concatenate([part[S:], part[:S]], axis=0)
        acc += part
    return acc.reshape(B, S, H)



# revision 3
# speedup vs baseline: 1.0667x; 1.0667x over previous
"""Trainium2 Bass kernel: GQA causal attention (B=2, S=2048, H=2048, 16 q-heads,
4 kv-heads, head_dim=128), 2-D sharded over 8 NeuronCores.

Sharding: batch x kv-group.  Core c owns batch g=c//4 and kv-head kv=c%4 with
its 4 GQA q-heads [4kv, 4kv+4).  Every projection is local (no collectives);
the host sums the 4 o_proj partials per batch (standard TP partial-sum
unshard, free on host).

Projections (q/k/v/o) run as fp8e4 DoubleRow matmuls at 2 contraction rows
per PE cell.  To keep bf16-grade accuracy each operand is split hi/lo into
two fp8 values (x = x_hi + x_lo captures ~11 mantissa bits); the product
uses the 3-term expansion

    x @ w  ~=  (x_hi + x_lo) @ w_hi  +  x_hi @ w_lo      (lo*lo dropped)

where term 1 pairs {x_hi, x_lo} in the DoubleRow k-slot against a
duplicated (stride-0) w_hi, and term 2 pairs adjacent 128-deep h-chunks.
Net PE cost is 0.75x of bf16 for K>=256 projections.  fp8 needs value
ranges centered in e4m3's [2^-6, 240]: x is pre-scaled by 16, weights by
512 (wq by 4096, absorbing 1/sqrt(D)); the compensations fold into the
q rope tables (/2^29), the rowsum ones constant (512 instead of 1), and a
final host-side divide by 8192.  Validated numerically: rel err 4.7e-3 vs
the fp32 reference (better than all-bf16's 5.4e-3).

Attention (scores / P@V / rowsum) stays bf16: its per-tile contraction is
only 128 deep, so DoubleRow pairing buys nothing without a lossy single-fp8
operand (measured 3e-2+ rel err -- over the gate).

On-chip layouts are transposed (feature-on-partition) except V:
  q/k:   qkvT = w.T @ x.T            (PE DoubleRow, fp8 hi/lo)
  v:     natural [s, d] directly     (PE DoubleRow, x stationary)
  RoPE:  q' = q*cos + rot(q)*sin     (DVE, sign-folded sin table)
  scoresT[k,q] = K @ Q^T             (PE bf16)
  P^T   = exp(scoresT - 40)          (ACT; exact softmax after norm)
  causal mask: affine_select on P^T  (Pool, fill=0)
  outT  = V^T @ P^T                  (PE bf16, accumulated over k-chunks)
  rowsum: quad-packed ones-matmuls   (DVE pre-sums quads; ones = 512.0)
  outT16 = o_ps * (1/rowsum')        (DVE recip+mul -> 16x attn-out bf16)
  hi/lo fp8 split of outT16          (Pool copy + sub; bit-exact pair)
  out'  = wo8^T-stationary DoubleRow (PE fp8; output [feat, row], host
                                      transposes + sums + /8192)

o_proj runs as a drip FIFO interleaved into the attention j-loops so the
in-order PE queue always has filler while the exp/mask chain (ACT/Pool) is
the critical path of the diagonal band.
"""

import os
import sys
import time

import numpy as np

sys.path.insert(0, "/opt/trn_rl_repo")

from contextlib import ExitStack

import concourse.bass as bass
from concourse import bacc
import concourse.mybir as mybir
import concourse.tile as tile
from concourse.bass_utils import run_bass_kernel_spmd

F32 = mybir.dt.float32
BF16 = mybir.dt.bfloat16
F8 = mybir.dt.float8e4
AF = mybir.ActivationFunctionType
ALU = mybir.AluOpType
PM = mybir.MatmulPerfMode.DoubleRow

B, S, H = 2, 2048, 2048
NH, KVH, D = 16, 4, 128
NCORES = 8
HPC = NH // KVH  # q heads per core = 4
SB = S // 512  # 4 row-blocks of 512
NB_HC = H // 128  # 16 contraction chunks
SC = S // 128  # 16 k-chunks
EXP_BIAS = -40.0

# fp8 scale plan (powers of two; compensated exactly)
SX = 16.0  # x pre-scale
SWQ = 4096.0  # wq pre-scale (includes 1/sqrt(D) folded separately)
SW = 512.0  # wk/wv/wo pre-scale
SA = 16.0  # attn-out pre-scale (folded into rowsum reciprocal)
OUT_DIV = 8192.0  # host divide: SA*SW

# w8 per-chunk column layout (CW wide)
CW = 1664
QHI, KHI, QLO, KLO, VHI, VHI2, VLO = 0, 512, 640, 1152, 1280, 1408, 1536
XW = 1024  # x8 per-chunk [hi 512 | lo 512]

LAST_EXEC_TIME_NS = None
LAST_RESULTS = None


def build_graph(reps=1):
    nc = bacc.Bacc(
        "TRN2", target_bir_lowering=False, debug=False, num_devices=NCORES
    )
    xTr8 = nc.dram_tensor("xTr8", [SB * 128, NB_HC * XW], F8, kind="ExternalInput").ap()
    w8d = nc.dram_tensor("w8d", [128, NB_HC * CW], F8, kind="ExternalInput").ap()
    wo8d = nc.dram_tensor("wo8d", [128, HPC * 4096], F8, kind="ExternalInput").ap()
    cosqd = nc.dram_tensor("cosqd", [D, S], BF16, kind="ExternalInput").ap()
    sinqd = nc.dram_tensor("sinqd", [D, S], BF16, kind="ExternalInput").ap()
    coskd = nc.dram_tensor("coskd", [D, S], BF16, kind="ExternalInput").ap()
    sinkd = nc.dram_tensor("sinkd", [D, S], BF16, kind="ExternalInput").ap()
    # out'[feat, s] = 8192 * (attn_out @ wo partial); host transposes/sums
    outp = nc.dram_tensor("outp", [H, S], BF16, kind="ExternalOutput").ap()

    with tile.TileContext(nc) as tc, ExitStack() as ctx:
        const_pool = ctx.enter_context(tc.tile_pool(name="const", bufs=1))
        w8 = const_pool.tile([128, NB_HC * CW], F8)
        wo8 = const_pool.tile([128, HPC * 4096], F8)
        cosq_sb = const_pool.tile([128, S], BF16)
        sinq_sb = const_pool.tile([128, S], BF16)
        cosk_sb = const_pool.tile([128, S], BF16)
        sink_sb = const_pool.tile([128, S], BF16)
        ones_sb = const_pool.tile([128, 128], BF16)  # rowsum lhsT; value 512
        expb_sb = const_pool.tile([128, 1], F32)
        scr_sb = const_pool.tile([128, 1], F32)
        # per-(head, qb) roped q tiles: fine-grained deps
        qk_sb = {
            (cg, rb): const_pool.tile([128, 512], BF16, name=f"qk{cg}_{rb}")
            for cg in range(HPC)
            for rb in range(SB)
        }
        kT_sb = const_pool.tile([128, S], BF16)  # roped kT (x8192)
        vall_sb = const_pool.tile([128, S], BF16)  # natural V (x8192)
        outT8 = const_pool.tile([128, HPC * 4096], F8)  # per head: [hi 2048|lo 2048]

        nc.gpsimd.memset(ones_sb[:], SW)  # 512: folds wo-scale into rowsum
        nc.gpsimd.memset(expb_sb[:], EXP_BIAS)
        # preheat the ACT Exp table while ACT is idle
        nc.scalar.activation(scr_sb[:], expb_sb[:], AF.Exp, bias=0.0, scale=1.0)

        xt_pool = ctx.enter_context(tc.tile_pool(name="xt", bufs=2))
        rtmp_pool = ctx.enter_context(tc.tile_pool(name="rtmp", bufs=8))
        pt_pool = ctx.enter_context(tc.tile_pool(name="pt", bufs=14))
        s2_pool = ctx.enter_context(tc.tile_pool(name="s2", bufs=6))
        s4_pool = ctx.enter_context(tc.tile_pool(name="s4", bufs=8))
        rr_sb_pool = ctx.enter_context(tc.tile_pool(name="rr_sb", bufs=2))
        ot16_pool = ctx.enter_context(tc.tile_pool(name="ot16", bufs=4))
        osb_pool = ctx.enter_context(tc.tile_pool(name="osb", bufs=6))
        op_ps_pool = ctx.enter_context(tc.tile_pool(name="op_ps", bufs=2, space="PSUM"))

        wo3 = {}  # (ch) -> [128, 2, 2048] hi|lo pair view of chunk ch
        wo3p = {}  # (cp) -> [128, 2, 4096] chunk-pair view
        oT3 = {}
        oT3p = {}
        for ch in range(HPC):
            wo3[ch] = wo8[:, ch * 4096 : (ch + 1) * 4096].rearrange(
                "p (two n) -> p two n", two=2
            )
            oT3[ch] = outT8[:, ch * 4096 : (ch + 1) * 4096].rearrange(
                "p (two n) -> p two n", two=2
            )
        for cp in range(HPC // 2):
            wo3p[cp] = wo8[:, cp * 8192 : (cp + 1) * 8192].rearrange(
                "p (two n) -> p two n", two=2
            )
            oT3p[cp] = outT8[:, cp * 8192 : (cp + 1) * 8192].rearrange(
                "p (two n) -> p two n", two=2
            )

        # ---- o_proj drip FIFO: unit = (qb, f) one [128 feat, 512 row] tile
        pending = []
        ncopy = [0]

        def emit_op(nmax, split=False, defer_below=0):
            for _ in range(nmax):
                if len(pending) <= defer_below:
                    return
                qb, f = pending.pop(0)
                op_ps = op_ps_pool.tile([128, 512], F32, tag="op", name="op_ps")
                for rhalf in range(2):
                    cols = slice(rhalf * 256, (rhalf + 1) * 256)
                    q0 = qb * 512 + rhalf * 256
                    for ch in range(HPC):
                        lhsT = wo8[:, ch * 4096 + f * 128 : ch * 4096 + (f + 1) * 128]
                        lhsT = lhsT.unsqueeze(1).broadcast_to([128, 2, 128])
                        nc.tensor.matmul(
                            op_ps[:, cols],
                            lhsT,
                            oT3[ch][:, :, q0 : q0 + 256],
                            start=(ch == 0),
                            stop=False,
                            perf_mode=PM,
                        )
                    for cp in range(HPC // 2):
                        nc.tensor.matmul(
                            op_ps[:, cols],
                            wo3p[cp][:, :, 2048 + f * 128 : 2048 + (f + 1) * 128],
                            oT3p[cp][:, :, q0 : q0 + 256],
                            start=False,
                            stop=(cp == HPC // 2 - 1),
                            perf_mode=PM,
                        )
                osb = osb_pool.tile([128, 512], BF16, tag="osb", name="osb")
                if split:
                    if ncopy[0] % 2 == 0:
                        nc.vector.tensor_copy(osb[:], op_ps[:])
                    else:
                        nc.scalar.copy(osb[:], op_ps[:])
                else:
                    if ncopy[0] % 3 == 1:
                        nc.scalar.copy(osb[:], op_ps[:])
                    else:
                        nc.vector.tensor_copy(osb[:], op_ps[:])
                ncopy[0] += 1
                nc.sync.dma_start(
                    outp[f * 128 : (f + 1) * 128, qb * 512 : (qb + 1) * 512], osb[:]
                )

        xts = {}

        def fetch(row0, key, granularity=2):
            t = xt_pool.tile([128, NB_HC * XW], F8, tag="xt")
            step = NB_HC // granularity
            for g in range(granularity):
                sl = slice(g * step * XW, (g + 1) * step * XW)
                nc.sync.dma_start(t[:, sl], xTr8[row0 : row0 + 128, sl])
            xts[key] = t

        def rope(ps, raw, dst, pos, cos_t, sin_t):
            # q' = q*cos + rot(q)*sin; sin table has rotate_half's sign folded
            # (rows 0:64 negated).  rot reads come partition-offset from PSUM.
            cs = cos_t[:, pos * 512 : (pos + 1) * 512]
            sn_lo = sin_t[0:64, pos * 512 : (pos + 1) * 512]
            sn_hi = sin_t[64:128, pos * 512 : (pos + 1) * 512]
            t1 = rtmp_pool.tile([128, 512], BF16, tag="rtmp")
            t2 = rtmp_pool.tile([128, 512], BF16, tag="rtmp")
            nc.vector.tensor_mul(t2[0:64, :], ps[64:128, :], sn_lo)
            nc.vector.tensor_mul(t2[64:128, :], ps[0:64, :], sn_hi)
            nc.vector.tensor_mul(t1[:], raw[:], cs)
            nc.vector.tensor_add(dst, t1[:], t2[:])

        def x3_main(xt, c, half):
            # moving {x_hi[c], x_lo[c]}: [128, 2, 256]
            return xt[:, c * XW : (c + 1) * XW].rearrange(
                "p (two n) -> p two n", two=2
            )[:, :, half * 256 : (half + 1) * 256]

        def x3_pair(xt, cp, lo, w):
            # moving {x_hi[2cp], x_hi[2cp+1]} sliced to [lo, lo+w)
            return xt[:, 2 * cp * XW : (2 * cp + 2) * XW].rearrange(
                "p (two n) -> p two n", two=2
            )[:, :, lo : lo + w]

        def w3_pair(cp, lo, width):
            return w8[:, 2 * cp * CW : (2 * cp + 2) * CW].rearrange(
                "p (two n) -> p two n", two=2
            )[:, :, lo : lo + width]

        def w_dup(c, off, width=128):
            return (
                w8[:, c * CW + off : c * CW + off + width]
                .unsqueeze(1)
                .broadcast_to([128, 2, width])
            )

        for _rep in range(reps):
            # ================= phase 1: projections (fp8 DoubleRow) ========
            with tc.tile_pool(name="proj_ps", bufs=6, space="PSUM") as proj_pool:
                for rb in range(SB):
                    if rb == 0:
                        # startup: first w/x chunks on separate queues so the
                        # first matmul's deps land early
                        t = xt_pool.tile([128, NB_HC * XW], F8, tag="xt")
                        xts[0] = t
                        for lo, hi in [(0, 1), (1, 2), (2, 4), (4, 6), (6, 8),
                                       (8, 10), (10, 12), (12, 14), (14, 16)]:
                            nc.sync.dma_start(
                                w8[:, lo * CW : hi * CW], w8d[:, lo * CW : hi * CW]
                            )
                            nc.scalar.dma_start(
                                t[:, lo * XW : hi * XW], xTr8[0:128, lo * XW : hi * XW]
                            )
                    xt = xts.pop(rb)
                    if rb + 1 < SB:
                        fetch((rb + 1) * 128, rb + 1)
                    if rb == 0:
                        nc.sync.dma_start(cosq_sb[:], cosqd)
                        nc.sync.dma_start(sinq_sb[:], sinqd)
                        nc.sync.dma_start(cosk_sb[:], coskd)
                        nc.sync.dma_start(sink_sb[:], sinkd)
                        nc.sync.dma_start(wo8[:], wo8d)
                    # --- q: two cg-pairs, halves sequential per psum bank ---
                    for cg0 in (0, 2):
                        qps = {
                            cg: proj_pool.tile([128, 512], F32, tag="pj", name=f"q{cg}")
                            for cg in (cg0, cg0 + 1)
                        }
                        for half in range(2):
                            cols = slice(half * 256, (half + 1) * 256)
                            for c in range(NB_HC):
                                for cg in (cg0, cg0 + 1):
                                    nc.tensor.matmul(
                                        qps[cg][:, cols],
                                        w_dup(c, QHI + cg * 128),
                                        x3_main(xt, c, half),
                                        start=(c == 0),
                                        stop=False,
                                        perf_mode=PM,
                                    )
                            for cp in range(NB_HC // 2):
                                for cg in (cg0, cg0 + 1):
                                    nc.tensor.matmul(
                                        qps[cg][:, cols],
                                        w3_pair(cp, QLO + cg * 128, 128),
                                        x3_pair(xt, cp, half * 256, 256),
                                        start=False,
                                        stop=(cp == NB_HC // 2 - 1),
                                        perf_mode=PM,
                                    )
                        for cg in (cg0, cg0 + 1):
                            raw = rtmp_pool.tile([128, 512], BF16, tag="rtmp")
                            nc.scalar.copy(raw[:], qps[cg][:])
                            rope(qps[cg], raw, qk_sb[(cg, rb)][:], rb,
                                 cosq_sb, sinq_sb)
                    # --- k ---
                    kps = proj_pool.tile([128, 512], F32, tag="pj", name="k")
                    for half in range(2):
                        cols = slice(half * 256, (half + 1) * 256)
                        for c in range(NB_HC):
                            nc.tensor.matmul(
                                kps[:, cols], w_dup(c, KHI), x3_main(xt, c, half),
                                start=(c == 0), stop=False, perf_mode=PM,
                            )
                        for cp in range(NB_HC // 2):
                            nc.tensor.matmul(
                                kps[:, cols],
                                w3_pair(cp, KLO, 128),
                                x3_pair(xt, cp, half * 256, 256),
                                start=False, stop=(cp == NB_HC // 2 - 1),
                                perf_mode=PM,
                            )
                    kraw = rtmp_pool.tile([128, 512], BF16, tag="rtmp")
                    nc.scalar.copy(kraw[:], kps[:])
                    rope(kps, kraw, kT_sb[:, rb * 512 : (rb + 1) * 512], rb,
                         cosk_sb, sink_sb)
                    # --- v (natural [s, d]; x stationary) ---
                    vps = proj_pool.tile([128, 512], F32, tag="pj", name="v")
                    for rc in range(4):
                        vcols = slice(rc * 128, (rc + 1) * 128)
                        for c in range(NB_HC):
                            lhsT = xt[:, c * XW : (c + 1) * XW].rearrange(
                                "p (two n) -> p two n", two=2
                            )[:, :, rc * 128 : (rc + 1) * 128]
                            nc.tensor.matmul(
                                vps[:, vcols],
                                lhsT,
                                w8[:, c * CW + VHI : c * CW + VHI + 256].rearrange(
                                    "p (two n) -> p two n", two=2
                                ),
                                start=(c == 0), stop=False, perf_mode=PM,
                            )
                        for cp in range(NB_HC // 2):
                            nc.tensor.matmul(
                                vps[:, vcols],
                                x3_pair(xt, cp, rc * 128, 128),
                                w3_pair(cp, VLO, 128),
                                start=False, stop=(cp == NB_HC // 2 - 1),
                                perf_mode=PM,
                            )
                    nc.scalar.copy(vall_sb[:, rb * 512 : (rb + 1) * 512], vps[:])

            # ================= phase 2: attention (bf16) ====================
            with (
                tc.tile_pool(name="rs_ps", bufs=2, space="PSUM") as rs_ps_pool,
                tc.tile_pool(name="ot_ps", bufs=2, space="PSUM") as ot_ps_pool,
                tc.tile_pool(name="st_ps", bufs=2, space="PSUM") as st_ps_pool,
            ):
                PD = 5
                for qb in range(SB):
                    nj = 4 * qb + 4
                    for pair in range(2):
                        heads = (2 * pair, 2 * pair + 1)
                        o_ps, r_ps = {}, {}
                        for h in heads:
                            o_ps[h] = ot_ps_pool.tile(
                                [128, 512], F32, tag="ot", name=f"ot{h}"
                            )
                            r_ps[h] = rs_ps_pool.tile(
                                [128, 512], F32, tag="rs", name=f"rs{h}"
                            )
                        pd = 3 if (qb == SB - 1 and pair == 1) else PD
                        pts, s2s, s4s, diag = {}, {}, {}, {}
                        for jj in range(nj + pd):
                            emit_op(3 if jj < 2 else 1)
                            if jj < nj:
                                j = jj
                                r = j - 4 * qb
                                qoff = 128 * r if r > 0 else 0
                                for h in heads:
                                    s_ps = st_ps_pool.tile([128, 512], F32)
                                    nc.tensor.matmul(
                                        s_ps[:, qoff:512],
                                        kT_sb[:, j * 128 : (j + 1) * 128],
                                        qk_sb[(h, qb)][:, qoff:512],
                                        start=True,
                                        stop=True,
                                    )
                                    pt = pt_pool.tile([128, 512], BF16)
                                    nc.scalar.activation(
                                        pt[:, qoff:512],
                                        s_ps[:, qoff:512],
                                        AF.Exp,
                                        bias=expb_sb[:],
                                        scale=1.0,
                                    )
                                    if r >= 0:
                                        nc.gpsimd.affine_select(
                                            out=pt[:, qoff : qoff + 128],
                                            in_=pt[:, qoff : qoff + 128],
                                            pattern=[[1, 128]],
                                            compare_op=ALU.is_ge,
                                            fill=0.0,
                                            base=0,
                                            channel_multiplier=-1,
                                        )
                                    pts[(h, j)] = (pt, qoff)
                                    padd = nc.vector.tensor_add
                                    pcopy = nc.vector.tensor_copy
                                    if j < 4 * qb:
                                        if j % 2 == 1:
                                            s2 = s2_pool.tile([128, 512], BF16, tag="s2")
                                            padd(s2[:], pts[(h, j - 1)][0][:], pt[:])
                                            s2s[(h, j // 2)] = s2
                                        if j % 4 == 3:
                                            s4 = s4_pool.tile([128, 512], BF16, tag="s4")
                                            padd(
                                                s4[:],
                                                s2s.pop((h, j // 2 - 1))[:],
                                                s2s.pop((h, j // 2))[:],
                                            )
                                            s4s[(h, j // 4)] = s4
                                    elif r == 1:
                                        pt0 = pts[(h, 4 * qb)][0]
                                        sa = s4_pool.tile([128, 512], BF16, tag="s4")
                                        pcopy(sa[:, 0:128], pt0[:, 0:128])
                                        padd(
                                            sa[:, 128:512],
                                            pt0[:, 128:512],
                                            pt[:, 128:512],
                                        )
                                        diag[(h, 0)] = sa
                                    elif r == 3:
                                        pt2 = pts[(h, 4 * qb + 2)][0]
                                        sb_ = s4_pool.tile([128, 512], BF16, tag="s4")
                                        pcopy(sb_[:, 256:384], pt2[:, 256:384])
                                        padd(
                                            sb_[:, 384:512],
                                            pt2[:, 384:512],
                                            pt[:, 384:512],
                                        )
                                        diag[(h, 1)] = sb_
                            if jj >= pd:
                                j2 = jj - pd
                                for h in heads:
                                    pt2, qoff2 = pts.pop((h, j2))
                                    if j2 < 4 * qb:
                                        if j2 % 4 == 3:
                                            s4c = s4s.pop((h, j2 // 4))
                                            nc.tensor.matmul(
                                                r_ps[h][:],
                                                ones_sb[:],
                                                s4c[:],
                                                start=(j2 == 3),
                                                stop=False,
                                                skip_group_check=True,
                                            )
                                    elif j2 == 4 * qb + 1:
                                        nc.tensor.matmul(
                                            r_ps[h][:],
                                            ones_sb[:],
                                            diag[(h, 0)][:],
                                            start=(qb == 0),
                                            stop=False,
                                            skip_group_check=True,
                                        )
                                    elif j2 == 4 * qb + 3:
                                        nc.tensor.matmul(
                                            r_ps[h][:, 256:512],
                                            ones_sb[:],
                                            diag[(h, 1)][:, 256:512],
                                            start=False,
                                            stop=True,
                                            skip_group_check=True,
                                        )
                                    nc.tensor.matmul(
                                        o_ps[h][:, qoff2:512],
                                        vall_sb[:, j2 * 128 : (j2 + 1) * 128],
                                        pt2[:, qoff2:512],
                                        start=(j2 == 0),
                                        stop=(j2 == nj - 1),
                                        skip_group_check=True,
                                    )
                            emit_op(1)
                        for h in heads:
                            rr = rr_sb_pool.tile([128, 512], F32, tag="rr")
                            nc.vector.reciprocal(rr[:], r_ps[h][:])
                            ot16 = ot16_pool.tile([128, 512], BF16, tag="ot16")
                            nc.vector.tensor_mul(ot16[:], o_ps[h][:], rr[:])
                            # hi/lo fp8 split on Pool (bit-exact pair)
                            nc.gpsimd.tensor_copy(
                                outT8[:, h * 4096 + qb * 512 : h * 4096 + (qb + 1) * 512],
                                ot16[:],
                            )
                            nc.gpsimd.tensor_sub(
                                outT8[:, h * 4096 + 2048 + qb * 512 :
                                      h * 4096 + 2048 + (qb + 1) * 512],
                                ot16[:],
                                outT8[:, h * 4096 + qb * 512 : h * 4096 + (qb + 1) * 512],
                            )
                    for f in range(16):
                        pending.append((qb, f))
            # final drain
            with tc.tile_pool(name="drain_ps", bufs=4, space="PSUM"):
                emit_op(len(pending), split=True)
    nc.compile()
    return nc


_GRAPH = None


def _rope_tables():
    inv_freq = 1.0 / (10000.0 ** (np.arange(0, D, 2, dtype=np.float32) / D))
    t = np.arange(S, dtype=np.float32)
    freqs = np.outer(t, inv_freq)
    emb = np.concatenate([freqs, freqs], axis=-1)  # (S, D)
    cosT = np.ascontiguousarray(np.cos(emb).T.astype(np.float32))
    sinT = np.ascontiguousarray(np.sin(emb).T.astype(np.float32))
    sinadjT = sinT.copy()
    sinadjT[0:64, :] *= -1.0
    return cosT, sinadjT


def _split8(a, f8):
    hi = a.astype(f8)
    lo = (a - hi.astype(np.float32)).astype(f8)
    return hi, lo


def kernel(x, wq, wk, wv, wo):
    global _GRAPH, LAST_EXEC_TIME_NS, LAST_RESULTS
    import ml_dtypes

    bf16 = ml_dtypes.bfloat16
    f8 = ml_dtypes.float8_e4m3
    x = np.asarray(x, dtype=np.float32)
    wq = np.asarray(wq, dtype=np.float32)
    wk = np.asarray(wk, dtype=np.float32)
    wv = np.asarray(wv, dtype=np.float32)
    wo = np.asarray(wo, dtype=np.float32)

    invD = np.float32(1.0 / np.sqrt(D))
    cosT, sinadjT = _rope_tables()
    QTS = np.float32(1.0 / (SX * SWQ * SX * SW))  # q tables: /(2^16 * 2^13)
    # q-psum = SX*SWQ*(x@wq/sqrt(D)); roped-q must equal true/(SX*SW) so that
    # scores = qk * (SX*SW * k-true) come out exact:
    #   qk = qps * cos * QTS = qr / 8192
    cosq = (cosT * QTS).astype(bf16)
    sinq = (sinadjT * QTS).astype(bf16)
    cosk = cosT.astype(bf16)
    sink = sinadjT.astype(bf16)

    # x: per batch-group, fp8 hi/lo chunk-packed
    xg8 = []
    for g in range(B):
        xT = np.ascontiguousarray(x[g].T) * SX  # [H, S]
        xh, xl = _split8(xT, f8)
        xh_r = xh.reshape(NB_HC, 128, SB, 512)
        xl_r = xl.reshape(NB_HC, 128, SB, 512)
        packed = np.stack([xh_r, xl_r], axis=3)  # [hc, p, rb, sel, col]
        xg8.append(
            np.ascontiguousarray(
                packed.transpose(2, 1, 0, 3, 4).reshape(SB * 128, NB_HC * XW)
            )
        )

    w8s, wo8s = [], []
    for kv in range(KVH):
        wq_c = wq[:, kv * HPC * D : (kv + 1) * HPC * D] * (invD * np.float32(SWQ))
        wk_c = wk[:, kv * D : (kv + 1) * D] * np.float32(SW)
        wv_c = wv[:, kv * D : (kv + 1) * D] * np.float32(SW)
        qh, ql = _split8(wq_c, f8)
        kh, kl = _split8(wk_c, f8)
        vh, vl = _split8(wv_c, f8)
        secs = [qh, kh, ql, kl, vh, vh, vl]
        chunk = np.concatenate(
            [s.reshape(NB_HC, 128, -1) for s in secs], axis=2
        )  # [hc, 128, CW]
        w8s.append(
            np.ascontiguousarray(chunk.transpose(1, 0, 2).reshape(128, NB_HC * CW))
        )
        wo_c = wo[kv * HPC * D : (kv + 1) * HPC * D, :] * np.float32(SW)
        oh, ol = _split8(wo_c, f8)
        blk = np.concatenate(
            [oh.reshape(HPC, 128, H), ol.reshape(HPC, 128, H)], axis=2
        )  # [ch, 128, 4096]
        wo8s.append(
            np.ascontiguousarray(blk.transpose(1, 0, 2).reshape(128, HPC * 4096))
        )

    in_maps = []
    for c in range(NCORES):
        g, kv = c // KVH, c % KVH
        in_maps.append(
            {
                "xTr8": xg8[g],
                "w8d": w8s[kv],
                "wo8d": wo8s[kv],
                "cosqd": cosq,
                "sinqd": sinq,
                "coskd": cosk,
                "sinkd": sink,
            }
        )

    if _GRAPH is None:
        _GRAPH = build_graph()

    os.environ["BASS_NEVER_TRACE"] = "1"
    res = None
    for attempt in range(3):
        try:
            res = run_bass_kernel_spmd(
                _GRAPH, in_maps, core_ids=list(range(NCORES))
            )
            break
        except Exception:
            if attempt == 2:
                raise
            time.sleep(5.0)
    LAST_EXEC_TIME_NS = res.exec_time_ns
    LAST_RESULTS = res
    out = np.zeros((B, S, H), dtype=np.float32)
    for c in range(NCORES):
        g = c // KVH
        out[g] += np.asarray(res.results[c]["outp"], dtype=np.float32).T
    out *= np.float32(1.0 / OUT_DIV)
    return out


# revision 14
# speedup vs baseline: 1.0924x; 1.0241x over previous
"""Trainium2 Bass kernel: GQA causal attention (B=2, S=2048, H=2048, 16 q-heads,
4 kv-heads, head_dim=128), 2-D sharded over 8 NeuronCores.

Sharding: batch x kv-group.  Core c owns batch g=c//4 and kv-head kv=c%4 with
its 4 GQA q-heads [4kv, 4kv+4).  Every projection is local (no collectives);
the host sums the 4 o_proj partials per batch (standard TP partial-sum
unshard, free on host).

Projections (q/k/v/o) run as fp8e4 DoubleRow matmuls at 2 contraction rows
per PE cell.  To keep bf16-grade accuracy each operand is split hi/lo into
two fp8 values (x = x_hi + x_lo captures ~11 mantissa bits); the product
uses the 3-term expansion

    x @ w  ~=  (x_hi + x_lo) @ w_hi  +  x_hi @ w_lo      (lo*lo dropped)

where term 1 pairs {x_hi, x_lo} in the DoubleRow k-slot against a
duplicated (stride-0) w_hi, and term 2 pairs adjacent 128-deep h-chunks.
Net PE cost is 0.75x of bf16 for K>=256 projections.  fp8 needs value
ranges centered in e4m3's [2^-6, 240]: x is pre-scaled by 16, weights by
512 (wq by 4096, absorbing 1/sqrt(D)); the compensations fold into the
q rope tables (/2^29), the rowsum ones constant (512 instead of 1), and a
final host-side divide by 8192.  Validated numerically: rel err 5.3e-3 vs
the fp32 reference (better than all-bf16's 6.6e-3).

Attention (scores / P@V / rowsum) stays bf16: its per-tile contraction is
only 128 deep, so DoubleRow pairing buys nothing without a lossy single-fp8
operand (measured 3e-2+ rel err -- over the gate).

On-chip layouts are transposed (feature-on-partition) except V:
  q/k:   qkvT = w.T @ x.T            (PE DoubleRow, fp8 hi/lo)
  v:     natural [s, d] directly     (PE DoubleRow, x stationary)
  RoPE:  q' = q*cos + rot(q)*sin     (DVE, sign-folded sin table)
  scoresT[k,q] = K @ Q^T             (PE bf16)
  P^T   = exp(scoresT - 40)          (ACT; exact softmax after norm)
  causal mask: P^T *= tri-mask       (DVE mul with a const mask tile --
                                      keeps the Pool queue free of
                                      head-of-line blocking)
  outT  = V^T @ P^T                  (PE bf16, accumulated over k-chunks)
  rowsum: quad-packed ones-matmuls   (DVE pre-sums quads; ones = 512.0)
  outT16 = o_ps * (1/rowsum')        (DVE recip+mul -> 16x attn-out bf16)
  hi/lo fp8 split of outT16          (Pool/DVE alternating copy + sub)
  out'  = wo8^T-stationary DoubleRow (PE fp8; output [feat, row], host
                                      transposes + sums + /8192)

Scheduling: the PE queue is in-order, so emission order is the schedule.
o_proj runs as a drip FIFO interleaved into the attention j-loops; each
attention pair's post-processing (recip/mul/fp8-split) is deferred into the
next pair's j-loop; qb0's attention is interleaved into rb3's k/v
projections (it only depends on rb0); startup DMAs are split across queues
with only w8+x(rb0) on the critical path; the last qb's posts run at
half-width so the tail drain starts sooner.
"""

import os
import sys
import time

import numpy as np

sys.path.insert(0, "/opt/trn_rl_repo")

from contextlib import ExitStack

import concourse.bass as bass
from concourse import bacc
import concourse.mybir as mybir
import concourse.tile as tile
from concourse.bass_utils import run_bass_kernel_spmd

F32 = mybir.dt.float32
BF16 = mybir.dt.bfloat16
F8 = mybir.dt.float8e4
AF = mybir.ActivationFunctionType
ALU = mybir.AluOpType
PM = mybir.MatmulPerfMode.DoubleRow

B, S, H = 2, 2048, 2048
NH, KVH, D = 16, 4, 128
NCORES = 8
HPC = NH // KVH  # q heads per core = 4
SB = S // 512  # 4 row-blocks of 512
NB_HC = H // 128  # 16 contraction chunks
SC = S // 128  # 16 k-chunks
EXP_BIAS = -40.0

# fp8 scale plan (powers of two; compensated exactly)
SX = 16.0  # x pre-scale
SWQ = 4096.0  # wq pre-scale (1/sqrt(D) folded into the weights too)
SW = 512.0  # wk/wv/wo pre-scale
OUT_DIV = 8192.0  # host divide: 16 * 512

# w8 per-chunk column layout (CW wide)
CW = 1664
QHI, KHI, QLO, KLO, VHI, VHI2, VLO = 0, 512, 640, 1152, 1280, 1408, 1536
XW = 1024  # x8 per-chunk [hi 512 | lo 512]

LAST_EXEC_TIME_NS = None
LAST_RESULTS = None


def build_graph(reps=1):
    nc = bacc.Bacc(
        "TRN2", target_bir_lowering=False, debug=False, num_devices=NCORES
    )
    xTr8 = nc.dram_tensor("xTr8", [SB * 128, NB_HC * XW], F8, kind="ExternalInput").ap()
    w8d = nc.dram_tensor("w8d", [128, NB_HC * CW], F8, kind="ExternalInput").ap()
    wo8d = nc.dram_tensor("wo8d", [128, HPC * 4096], F8, kind="ExternalInput").ap()
    cosqd = nc.dram_tensor("cosqd", [D, S], BF16, kind="ExternalInput").ap()
    sinqd = nc.dram_tensor("sinqd", [D, S], BF16, kind="ExternalInput").ap()
    coskd = nc.dram_tensor("coskd", [D, S], BF16, kind="ExternalInput").ap()
    sinkd = nc.dram_tensor("sinkd", [D, S], BF16, kind="ExternalInput").ap()
    # out'[feat, s] = 8192 * (attn_out @ wo partial); host transposes/sums
    outp = nc.dram_tensor("outp", [H, S], BF16, kind="ExternalOutput").ap()

    with tile.TileContext(nc) as tc, ExitStack() as ctx:
        const_pool = ctx.enter_context(tc.tile_pool(name="const", bufs=1))
        w8 = const_pool.tile([128, NB_HC * CW], F8)
        wo8 = const_pool.tile([128, HPC * 4096], F8)
        cosq_sb = const_pool.tile([128, S], BF16)
        sinq_sb = const_pool.tile([128, S], BF16)
        cosk_sb = const_pool.tile([128, S], BF16)
        sink_sb = const_pool.tile([128, S], BF16)
        ones_sb = const_pool.tile([128, 128], BF16)  # rowsum lhsT; value 512
        mask_sb = const_pool.tile([128, 128], BF16)  # causal tri mask (col>=p)
        expb_sb = const_pool.tile([128, 1], F32)
        scr_sb = const_pool.tile([128, 1], F32)
        qk_sb = {
            (cg, rb): const_pool.tile([128, 512], BF16, name=f"qk{cg}_{rb}")
            for cg in range(HPC)
            for rb in range(SB)
        }
        kT_sb = const_pool.tile([128, S], BF16)  # roped kT (x8192)
        vall_sb = const_pool.tile([128, S], BF16)  # natural V (x8192)
        outT8 = const_pool.tile([128, HPC * 4096], F8)  # per head: [hi|lo]

        nc.gpsimd.memset(ones_sb[:], SW)  # 512: folds wo-scale into rowsum
        nc.gpsimd.memset(mask_sb[:], 1.0)
        nc.gpsimd.affine_select(
            out=mask_sb[:], in_=mask_sb[:], pattern=[[1, 128]],
            compare_op=ALU.is_ge, fill=0.0, base=0, channel_multiplier=-1,
        )
        nc.gpsimd.memset(expb_sb[:], EXP_BIAS)
        # preheat the ACT Exp table while ACT is idle
        nc.scalar.activation(scr_sb[:], expb_sb[:], AF.Exp, bias=0.0, scale=1.0)

        xt_pool = ctx.enter_context(tc.tile_pool(name="xt", bufs=2))
        rtmp_pool = ctx.enter_context(tc.tile_pool(name="rtmp", bufs=8))
        pt_pool = ctx.enter_context(tc.tile_pool(name="pt", bufs=14))
        s2_pool = ctx.enter_context(tc.tile_pool(name="s2", bufs=6))
        s4_pool = ctx.enter_context(tc.tile_pool(name="s4", bufs=8))
        rr_sb_pool = ctx.enter_context(tc.tile_pool(name="rr_sb", bufs=4))
        ot16_pool = ctx.enter_context(tc.tile_pool(name="ot16", bufs=4))
        osb_pool = ctx.enter_context(tc.tile_pool(name="osb", bufs=6))

        wo3, wo3p, oT3, oT3p = {}, {}, {}, {}
        for ch in range(HPC):
            wo3[ch] = wo8[:, ch * 4096 : (ch + 1) * 4096].rearrange(
                "p (two n) -> p two n", two=2
            )
            oT3[ch] = outT8[:, ch * 4096 : (ch + 1) * 4096].rearrange(
                "p (two n) -> p two n", two=2
            )
        for cp in range(HPC // 2):
            wo3p[cp] = wo8[:, cp * 8192 : (cp + 1) * 8192].rearrange(
                "p (two n) -> p two n", two=2
            )
            oT3p[cp] = outT8[:, cp * 8192 : (cp + 1) * 8192].rearrange(
                "p (two n) -> p two n", two=2
            )

        # ---- o_proj drip FIFO: unit = (qb, f) one [128 feat, 512 row] tile
        pending = []
        ncopy = [0]
        op_pool_ref = [None]

        def emit_op(nmax, split=False):
            for _ in range(nmax):
                if not pending:
                    return
                qb, f = pending.pop(0)
                op_ps = op_pool_ref[0].tile([128, 512], F32, tag="op", name="op_ps")
                for rhalf in range(2):
                    cols = slice(rhalf * 256, (rhalf + 1) * 256)
                    q0 = qb * 512 + rhalf * 256
                    for ch in range(HPC):
                        lhsT = wo8[:, ch * 4096 + f * 128 : ch * 4096 + (f + 1) * 128]
                        lhsT = lhsT.unsqueeze(1).broadcast_to([128, 2, 128])
                        nc.tensor.matmul(
                            op_ps[:, cols],
                            lhsT,
                            oT3[ch][:, :, q0 : q0 + 256],
                            start=(ch == 0),
                            stop=False,
                            perf_mode=PM,
                        )
                    for cp in range(HPC // 2):
                        nc.tensor.matmul(
                            op_ps[:, cols],
                            wo3p[cp][:, :, 2048 + f * 128 : 2048 + (f + 1) * 128],
                            oT3p[cp][:, :, q0 : q0 + 256],
                            start=False,
                            stop=(cp == HPC // 2 - 1),
                            perf_mode=PM,
                        )
                osb = osb_pool.tile([128, 512], BF16, tag="osb", name="osb")
                if split:
                    # tail drain: per-half copies on both engines shorten the
                    # last copy->DMA chain
                    nc.vector.tensor_copy(osb[:, 0:256], op_ps[:, 0:256])
                    nc.scalar.copy(osb[:, 256:512], op_ps[:, 256:512])
                else:
                    if ncopy[0] % 2 == 1:
                        nc.scalar.copy(osb[:], op_ps[:])
                    else:
                        nc.vector.tensor_copy(osb[:], op_ps[:])
                ncopy[0] += 1
                nc.sync.dma_start(
                    outp[f * 128 : (f + 1) * 128, qb * 512 : (qb + 1) * 512], osb[:]
                )

        xts = {}

        def fetch(row0, key, granularity=2):
            t = xt_pool.tile([128, NB_HC * XW], F8, tag="xt")
            step = NB_HC // granularity
            for g in range(granularity):
                sl = slice(g * step * XW, (g + 1) * step * XW)
                nc.sync.dma_start(t[:, sl], xTr8[row0 : row0 + 128, sl])
            xts[key] = t

        def rope(ps, raw, dst, pos, cos_t, sin_t):
            # q' = q*cos + rot(q)*sin; sin table has rotate_half's sign folded
            cs = cos_t[:, pos * 512 : (pos + 1) * 512]
            sn_lo = sin_t[0:64, pos * 512 : (pos + 1) * 512]
            sn_hi = sin_t[64:128, pos * 512 : (pos + 1) * 512]
            t1 = rtmp_pool.tile([128, 512], BF16, tag="rtmp")
            t2 = rtmp_pool.tile([128, 512], BF16, tag="rtmp")
            nc.vector.tensor_mul(t2[0:64, :], ps[64:128, :], sn_lo)
            nc.vector.tensor_mul(t2[64:128, :], ps[0:64, :], sn_hi)
            nc.vector.tensor_mul(t1[:], raw[:], cs)
            nc.vector.tensor_add(dst, t1[:], t2[:])

        def x3_main(xt, c, half):
            return xt[:, c * XW : (c + 1) * XW].rearrange(
                "p (two n) -> p two n", two=2
            )[:, :, half * 256 : (half + 1) * 256]

        def x3_pair(xt, cp, lo, w):
            return xt[:, 2 * cp * XW : (2 * cp + 2) * XW].rearrange(
                "p (two n) -> p two n", two=2
            )[:, :, lo : lo + w]

        def w3_pair(cp, lo, width):
            return w8[:, 2 * cp * CW : (2 * cp + 2) * CW].rearrange(
                "p (two n) -> p two n", two=2
            )[:, :, lo : lo + width]

        def w_dup(c, off, width=128):
            return (
                w8[:, c * CW + off : c * CW + off + width]
                .unsqueeze(1)
                .broadcast_to([128, 2, width])
            )

        # ---------- projection emitters (usable inline or as fill closures)
        def emit_q_pair(pool, xt, rb, cg0):
            qps = {
                cg: pool.tile([128, 512], F32, tag="pj", name=f"q{cg}")
                for cg in (cg0, cg0 + 1)
            }
            for half in range(2):
                cols = slice(half * 256, (half + 1) * 256)
                for c in range(NB_HC):
                    for cg in (cg0, cg0 + 1):
                        nc.tensor.matmul(
                            qps[cg][:, cols],
                            w_dup(c, QHI + cg * 128),
                            x3_main(xt, c, half),
                            start=(c == 0), stop=False, perf_mode=PM,
                        )
                for cp in range(NB_HC // 2):
                    for cg in (cg0, cg0 + 1):
                        nc.tensor.matmul(
                            qps[cg][:, cols],
                            w3_pair(cp, QLO + cg * 128, 128),
                            x3_pair(xt, cp, half * 256, 256),
                            start=False, stop=(cp == NB_HC // 2 - 1),
                            perf_mode=PM,
                        )
            for cg in (cg0, cg0 + 1):
                raw = rtmp_pool.tile([128, 512], BF16, tag="rtmp")
                nc.scalar.copy(raw[:], qps[cg][:])
                rope(qps[cg], raw, qk_sb[(cg, rb)][:], rb, cosq_sb, sinq_sb)

        def k_steps(pool, xt, rb):
            # yields closures: fine-grained emission units for interleaving
            kps = [None]

            def alloc():
                kps[0] = pool.tile([128, 512], F32, tag="kv", name="k")

            yield alloc
            for half in range(2):
                cols = slice(half * 256, (half + 1) * 256)
                for c0 in (0, 4, 8, 12):
                    def main(half=half, cols=cols, c0=c0):
                        for c in range(c0, c0 + 4):
                            nc.tensor.matmul(
                                kps[0][:, cols], w_dup(c, KHI), x3_main(xt, c, half),
                                start=(c == 0), stop=False, perf_mode=PM,
                            )
                    yield main
                for g0 in (0, 4):
                    def corr(half=half, cols=cols, g0=g0):
                        for cp in range(g0, g0 + 4):
                            nc.tensor.matmul(
                                kps[0][:, cols],
                                w3_pair(cp, KLO, 128),
                                x3_pair(xt, cp, half * 256, 256),
                                start=False, stop=(cp == NB_HC // 2 - 1),
                                perf_mode=PM,
                            )
                    yield corr

            def finish():
                kraw = rtmp_pool.tile([128, 512], BF16, tag="rtmp")
                nc.scalar.copy(kraw[:], kps[0][:])
                rope(kps[0], kraw, kT_sb[:, rb * 512 : (rb + 1) * 512], rb,
                     cosk_sb, sink_sb)
            yield finish

        def v_steps(pool, xt, rb):
            vps = [None]

            def alloc():
                vps[0] = pool.tile([128, 512], F32, tag="kv", name="v")

            yield alloc
            for rc in range(4):
                for c0 in (0, 8):
                    def main(rc=rc, c0=c0):
                        vcols = slice(rc * 128, (rc + 1) * 128)
                        for c in range(c0, c0 + 8):
                            lhsT = xt[:, c * XW : (c + 1) * XW].rearrange(
                                "p (two n) -> p two n", two=2
                            )[:, :, rc * 128 : (rc + 1) * 128]
                            nc.tensor.matmul(
                                vps[0][:, vcols],
                                lhsT,
                                w8[:, c * CW + VHI : c * CW + VHI + 256].rearrange(
                                    "p (two n) -> p two n", two=2
                                ),
                                start=(c == 0), stop=False, perf_mode=PM,
                            )
                    yield main

                def corr(rc=rc):
                    vcols = slice(rc * 128, (rc + 1) * 128)
                    for cp in range(NB_HC // 2):
                        nc.tensor.matmul(
                            vps[0][:, vcols],
                            x3_pair(xt, cp, rc * 128, 128),
                            w3_pair(cp, VLO, 128),
                            start=False, stop=(cp == NB_HC // 2 - 1),
                            perf_mode=PM,
                        )
                yield corr

            def drain():
                nc.scalar.copy(vall_sb[:, rb * 512 : (rb + 1) * 512], vps[0][:])
            yield drain

        # ---------- attention pair emitter ----------
        post_q = []  # deferred post-processing closures

        def make_post(h, qb, o_ps_h, r_ps_h):
            def post():
                # halves: o_proj units unblock per 256-col half via subtile
                # deps, so the first drip only waits ~half the split chain.
                rr = rr_sb_pool.tile([128, 512], F32, tag="rr")
                nc.vector.reciprocal(rr[:], r_ps_h[:])
                ot16 = ot16_pool.tile([128, 512], BF16, tag="ot16")
                for w0, w1 in ((0, 256), (256, 512)):
                    cols = slice(w0, w1)
                    nc.vector.tensor_mul(ot16[:, cols], o_ps_h[:, cols], rr[:, cols])
                    hi = outT8[:, h * 4096 + qb * 512 + w0 : h * 4096 + qb * 512 + w1]
                    lo = outT8[:, h * 4096 + 2048 + qb * 512 + w0 :
                               h * 4096 + 2048 + qb * 512 + w1]
                    nc.vector.tensor_copy(hi, ot16[:, cols])
                    nc.gpsimd.tensor_sub(lo, ot16[:, cols], hi)
            return post

        def run_fill(fill, n):
            for _ in range(n):
                if fill:
                    fill.pop(0)()

        def attn_pair(qb, pair, ot_pool, rs_pool, st_pool, fill, last=False):
            heads = (2 * pair, 2 * pair + 1)
            o_ps, r_ps = {}, {}
            for h in heads:
                o_ps[h] = ot_pool.tile([128, 512], F32, tag="ot", name=f"ot{h}")
                r_ps[h] = rs_pool.tile([128, 512], F32, tag="rs", name=f"rs{h}")
            nj = 4 * qb + 4
            pd = 3 if last else 5
            pts, s2s, s4s, diag = {}, {}, {}, {}
            for jj in range(nj + pd):
                if jj < 2:
                    # run deferred posts of the previous pair first so the
                    # o_proj units they gate don't stall the PE
                    while post_q:
                        post_q.pop(0)()
                run_fill(fill, 1)
                # on a qb's first pair, hold the drip until the previous qb's
                # last posts (emitted just above) have had a few score-tiles'
                # time to finish -- an o_proj unit at the head of the in-order
                # PE queue would otherwise block the attention stream
                if pair == 1 or jj >= 3:
                    emit_op(2 if jj < 4 else 1)
                if jj < nj:
                    j = jj
                    r = j - 4 * qb
                    qoff = 128 * r if r > 0 else 0
                    for h in heads:
                        s_ps = st_pool.tile([128, 512], F32)
                        nc.tensor.matmul(
                            s_ps[:, qoff:512],
                            kT_sb[:, j * 128 : (j + 1) * 128],
                            qk_sb[(h, qb)][:, qoff:512],
                            start=True,
                            stop=True,
                        )
                        pt = pt_pool.tile([128, 512], BF16)
                        nc.scalar.activation(
                            pt[:, qoff:512],
                            s_ps[:, qoff:512],
                            AF.Exp,
                            bias=expb_sb[:],
                            scale=1.0,
                        )
                        if r >= 0:
                            nc.vector.tensor_mul(
                                pt[:, qoff : qoff + 128],
                                pt[:, qoff : qoff + 128],
                                mask_sb[:],
                            )
                        pts[(h, j)] = (pt, qoff)
                        padd = nc.vector.tensor_add
                        pcopy = nc.vector.tensor_copy
                        if j < 4 * qb:
                            if j % 2 == 1:
                                s2 = s2_pool.tile([128, 512], BF16, tag="s2")
                                padd(s2[:], pts[(h, j - 1)][0][:], pt[:])
                                s2s[(h, j // 2)] = s2
                            if j % 4 == 3:
                                s4 = s4_pool.tile([128, 512], BF16, tag="s4")
                                padd(
                                    s4[:],
                                    s2s.pop((h, j // 2 - 1))[:],
                                    s2s.pop((h, j // 2))[:],
                                )
                                s4s[(h, j // 4)] = s4
                        elif r == 1:
                            pt0 = pts[(h, 4 * qb)][0]
                            sa = s4_pool.tile([128, 512], BF16, tag="s4")
                            pcopy(sa[:, 0:128], pt0[:, 0:128])
                            padd(sa[:, 128:512], pt0[:, 128:512], pt[:, 128:512])
                            diag[(h, 0)] = sa
                        elif r == 3:
                            pt2 = pts[(h, 4 * qb + 2)][0]
                            sb_ = s4_pool.tile([128, 512], BF16, tag="s4")
                            pcopy(sb_[:, 256:384], pt2[:, 256:384])
                            padd(sb_[:, 384:512], pt2[:, 384:512], pt[:, 384:512])
                            diag[(h, 1)] = sb_
                if jj >= pd:
                    j2 = jj - pd
                    for h in heads:
                        pt2, qoff2 = pts.pop((h, j2))
                        if j2 < 4 * qb:
                            if j2 % 4 == 3:
                                s4c = s4s.pop((h, j2 // 4))
                                nc.tensor.matmul(
                                    r_ps[h][:],
                                    ones_sb[:],
                                    s4c[:],
                                    start=(j2 == 3),
                                    stop=False,
                                    skip_group_check=True,
                                )
                        elif j2 == 4 * qb + 1:
                            nc.tensor.matmul(
                                r_ps[h][:],
                                ones_sb[:],
                                diag[(h, 0)][:],
                                start=(qb == 0),
                                stop=False,
                                skip_group_check=True,
                            )
                        elif j2 == 4 * qb + 3:
                            nc.tensor.matmul(
                                r_ps[h][:, 256:512],
                                ones_sb[:],
                                diag[(h, 1)][:, 256:512],
                                start=False,
                                stop=True,
                                skip_group_check=True,
                            )
                        nc.tensor.matmul(
                            o_ps[h][:, qoff2:512],
                            vall_sb[:, j2 * 128 : (j2 + 1) * 128],
                            pt2[:, qoff2:512],
                            start=(j2 == 0),
                            stop=(j2 == nj - 1),
                            skip_group_check=True,
                        )
                run_fill(fill, 1)
                emit_op(1)
            for h in heads:
                post_q.append(make_post(h, qb, o_ps[h], r_ps[h]))

        for _rep in range(reps):
            # ======== phase 1: projections rb0-rb2 + rb3 q (fp8) ===========
            with tc.tile_pool(name="proj_ps", bufs=4, space="PSUM") as proj_pool:
                for rb in range(SB):
                    if rb == 0:
                        # startup: w8 + x(rb0) interleaved on two queues are
                        # the only critical loads; tables/wo8 stream later
                        t = xt_pool.tile([128, NB_HC * XW], F8, tag="xt")
                        xts[0] = t
                        for lo, hi in [(0, 1), (1, 2), (2, 4), (4, 6), (6, 8),
                                       (8, 10), (10, 12), (12, 14), (14, 16)]:
                            nc.sync.dma_start(
                                w8[:, lo * CW : hi * CW], w8d[:, lo * CW : hi * CW]
                            )
                            nc.scalar.dma_start(
                                t[:, lo * XW : hi * XW], xTr8[0:128, lo * XW : hi * XW]
                            )
                    xt = xts.pop(rb)
                    if rb + 1 < SB:
                        # rb1 at fine grain: it starts consuming chunk 0
                        # before the whole block lands
                        fetch((rb + 1) * 128, rb + 1, granularity=4 if rb == 0 else 2)
                    if rb == 0:
                        # tables after the rb1 prefetch: ropes tolerate
                        # latency (qk/kT consumers are far away), rb1 doesn't
                        nc.scalar.dma_start(cosq_sb[:], cosqd)
                        nc.scalar.dma_start(sinq_sb[:], sinqd)
                        nc.scalar.dma_start(cosk_sb[:], coskd)
                        nc.scalar.dma_start(sink_sb[:], sinkd)
                    if rb == 2:
                        nc.scalar.dma_start(wo8[:], wo8d)
                    for cg0 in (0, 2):
                        emit_q_pair(proj_pool, xt, rb, cg0)
                    if rb < SB - 1:
                        for step in k_steps(proj_pool, xt, rb):
                            step()
                        for step in v_steps(proj_pool, xt, rb):
                            step()
                    else:
                        xt_last = xt
            # ======== attention; qb0 interleaved with rb3 k/v ==============
            with (
                tc.tile_pool(name="rs_ps", bufs=2, space="PSUM") as rs_pool,
                tc.tile_pool(name="ot_ps", bufs=2, space="PSUM") as ot_pool,
                tc.tile_pool(name="st_ps", bufs=2, space="PSUM") as st_pool,
            ):
                with tc.tile_pool(name="kv_ps", bufs=2, space="PSUM") as kv_pool:
                    kl = list(k_steps(kv_pool, xt_last, SB - 1))
                    vl = list(v_steps(kv_pool, xt_last, SB - 1))
                    # alloc both psum tiles up front, then interleave the rest
                    kl[0]()
                    vl[0]()
                    fill = kl[1:] + vl[1:]
                    attn_pair(0, 0, ot_pool, rs_pool, st_pool, fill)
                    attn_pair(0, 1, ot_pool, rs_pool, st_pool, fill)
                    run_fill(fill, len(fill))
                    for f in range(16):
                        pending.append((0, f))
                with tc.tile_pool(name="op_ps", bufs=2, space="PSUM") as op_pool:
                    op_pool_ref[0] = op_pool
                    for qb in range(1, SB):
                        for pair in range(2):
                            attn_pair(
                                qb, pair, ot_pool, rs_pool, st_pool, [],
                                last=(qb == SB - 1 and pair == 1),
                            )
                        for f in range(16):
                            pending.append((qb, f))
                    while post_q:
                        post_q.pop(0)()
                    emit_op(len(pending) - 8)
            # final drain with more banks once attention psum is closed
            with tc.tile_pool(name="drain_ps", bufs=6, space="PSUM") as drain_pool:
                op_pool_ref[0] = drain_pool
                emit_op(len(pending), split=True)
    nc.compile()
    return nc


_GRAPH = None


def _rope_tables():
    inv_freq = 1.0 / (10000.0 ** (np.arange(0, D, 2, dtype=np.float32) / D))
    t = np.arange(S, dtype=np.float32)
    freqs = np.outer(t, inv_freq)
    emb = np.concatenate([freqs, freqs], axis=-1)  # (S, D)
    cosT = np.ascontiguousarray(np.cos(emb).T.astype(np.float32))
    sinT = np.ascontiguousarray(np.sin(emb).T.astype(np.float32))
    sinadjT = sinT.copy()
    sinadjT[0:64, :] *= -1.0
    return cosT, sinadjT


def _split8(a, f8):
    hi = a.astype(f8)
    lo = (a - hi.astype(np.float32)).astype(f8)
    return hi, lo


def kernel(x, wq, wk, wv, wo):
    global _GRAPH, LAST_EXEC_TIME_NS, LAST_RESULTS
    import ml_dtypes

    f8 = ml_dtypes.float8_e4m3
    bf16 = ml_dtypes.bfloat16
    x = np.asarray(x, dtype=np.float32)
    wq = np.asarray(wq, dtype=np.float32)
    wk = np.asarray(wk, dtype=np.float32)
    wv = np.asarray(wv, dtype=np.float32)
    wo = np.asarray(wo, dtype=np.float32)

    invD = np.float32(1.0 / np.sqrt(D))
    cosT, sinadjT = _rope_tables()
    # q-psum = SX*SWQ*(x@wq/sqrt(D)); roped q must equal true/(SX*SW) so that
    # scores = qk . (SX*SW * k-true) come out exact
    QTS = np.float32(1.0 / (SX * SWQ * SX * SW))
    cosq = (cosT * QTS).astype(bf16)
    sinq = (sinadjT * QTS).astype(bf16)
    cosk = cosT.astype(bf16)
    sink = sinadjT.astype(bf16)

    xg8 = []
    for g in range(B):
        xT = np.ascontiguousarray(x[g].T) * np.float32(SX)  # [H, S]
        xh, xl = _split8(xT, f8)
        xh_r = xh.reshape(NB_HC, 128, SB, 512)
        xl_r = xl.reshape(NB_HC, 128, SB, 512)
        packed = np.stack([xh_r, xl_r], axis=3)  # [hc, p, rb, sel, col]
        xg8.append(
            np.ascontiguousarray(
                packed.transpose(2, 1, 0, 3, 4).reshape(SB * 128, NB_HC * XW)
            )
        )

    w8s, wo8s = [], []
    for kv in range(KVH):
        wq_c = wq[:, kv * HPC * D : (kv + 1) * HPC * D] * (invD * np.float32(SWQ))
        wk_c = wk[:, kv * D : (kv + 1) * D] * np.float32(SW)
        wv_c = wv[:, kv * D : (kv + 1) * D] * np.float32(SW)
        qh, ql = _split8(wq_c, f8)
        kh, kl = _split8(wk_c, f8)
        vh, vl = _split8(wv_c, f8)
        secs = [qh, kh, ql, kl, vh, vh, vl]
        chunk = np.concatenate(
            [s.reshape(NB_HC, 128, -1) for s in secs], axis=2
        )  # [hc, 128, CW]
        w8s.append(
            np.ascontiguousarray(chunk.transpose(1, 0, 2).reshape(128, NB_HC * CW))
        )
        wo_c = wo[kv * HPC * D : (kv + 1) * HPC * D, :] * np.float32(SW)
        oh, ol = _split8(wo_c, f8)
        blk = np.concatenate(
            [oh.reshape(HPC, 128, H), ol.reshape(HPC, 128, H)], axis=2
        )  # [ch, 128, 4096]
        wo8s.append(
            np.ascontiguousarray(blk.transpose(1, 0, 2).reshape(128, HPC * 4096))
        )

    in_maps = []
    for c in range(NCORES):
        g, kv = c // KVH, c % KVH
        in_maps.append(
            {
                "xTr8": xg8[g],
                "w8d": w8s[kv],
                "wo8d": wo8s[kv],
                "cosqd": cosq,
                "sinqd": sinq,
                "coskd": cosk,
                "sinkd": sink,
            }
        )

    if _GRAPH is None:
        _GRAPH = build_graph()

    os.environ["BASS_NEVER_TRACE"] = "1"
    res = None
    for attempt in range(3):
        try:
            res = run_bass_kernel_spmd(
                _GRAPH, in_maps, core_ids=list(range(NCORES))
            )
            break
        except Exception:
            if attempt == 2:
                raise
            time.sleep(5.0)
    LAST_EXEC_TIME_NS = res.exec_time_ns
    LAST_RESULTS = res
    out = np.zeros((B, S, H), dtype=np.float32)
    for c in range(NCORES):
        g = c // KVH
        out[g] += np.asarray(res.results[c]["outp"], dtype=np.float32).T
    out *= np.float32(1.0 / OUT_DIV)
    return out


# revision 19
# speedup vs baseline: 1.0970x; 1.0042x over previous
"""Trainium2 Bass kernel: GQA causal attention (B=2, S=2048, H=2048, 16 q-heads,
4 kv-heads, head_dim=128), 2-D sharded over 8 NeuronCores.

Sharding: batch x kv-group.  Core c owns batch g=c//4 and kv-head kv=c%4 with
its 4 GQA q-heads [4kv, 4kv+4).  Every projection is local (no collectives);
the host sums the 4 o_proj partials per batch (standard TP partial-sum
unshard, free on host).

Projections (q/k/v/o) run as fp8e4 DoubleRow matmuls at 2 contraction rows
per PE cell.  To keep bf16-grade accuracy each operand is split hi/lo into
two fp8 values (x = x_hi + x_lo captures ~11 mantissa bits); the product
uses the 3-term expansion

    x @ w  ~=  (x_hi + x_lo) @ w_hi  +  x_hi @ w_lo      (lo*lo dropped)

where term 1 pairs {x_hi, x_lo} in the DoubleRow k-slot against a
duplicated (stride-0) w_hi, and term 2 pairs adjacent 128-deep h-chunks.
Net PE cost is 0.75x of bf16 for K>=256 projections.  fp8 needs value
ranges centered in e4m3's [2^-6, 240]: x is pre-scaled by 16, weights by
512 (wq by 4096, absorbing 1/sqrt(D)); the compensations fold into the
q rope tables (/2^29), the rowsum ones constant (512 instead of 1), and a
final host-side divide by 8192.  Validated numerically: rel err 5.3e-3 vs
the fp32 reference (better than all-bf16's 6.6e-3).

Attention (scores / P@V / rowsum) stays bf16: its per-tile contraction is
only 128 deep, so DoubleRow pairing buys nothing without a lossy single-fp8
operand (measured 3e-2+ rel err -- over the gate).

On-chip layouts are transposed (feature-on-partition) except V:
  q/k:   qkvT = w.T @ x.T            (PE DoubleRow, fp8 hi/lo)
  v:     natural [s, d] directly     (PE DoubleRow, x stationary)
  RoPE:  q' = q*cos + rot(q)*sin     (DVE, sign-folded sin table)
  scoresT[k,q] = K @ Q^T             (PE bf16)
  P^T   = exp(scoresT - 40)          (ACT; exact softmax after norm)
  causal mask: P^T *= tri-mask       (DVE mul with a const mask tile --
                                      keeps the Pool queue free of
                                      head-of-line blocking)
  outT  = V^T @ P^T                  (PE bf16, accumulated over k-chunks)
  rowsum: quad-packed ones-matmuls   (DVE pre-sums quads; ones = 512.0)
  outT16 = o_ps * (1/rowsum')        (DVE recip+mul -> 16x attn-out bf16)
  hi/lo fp8 split of outT16          (Pool/DVE alternating copy + sub)
  out'  = wo8^T-stationary DoubleRow (PE fp8; output [feat, row], host
                                      transposes + sums + /8192)

Scheduling: the PE queue is in-order, so emission order is the schedule.
o_proj runs as a drip FIFO interleaved into the attention j-loops; each
attention pair's post-processing (recip/mul/fp8-split) is deferred into the
next pair's j-loop; qb0's attention is interleaved into rb3's k/v
projections (it only depends on rb0); startup DMAs are split across queues
with only w8+x(rb0) on the critical path; the last qb's posts run at
half-width so the tail drain starts sooner.
"""

import os
import sys
import time

import numpy as np

sys.path.insert(0, "/opt/trn_rl_repo")

from contextlib import ExitStack

import concourse.bass as bass
from concourse import bacc
import concourse.mybir as mybir
import concourse.tile as tile
from concourse.bass_utils import run_bass_kernel_spmd

F32 = mybir.dt.float32
BF16 = mybir.dt.bfloat16
F8 = mybir.dt.float8e4
AF = mybir.ActivationFunctionType
ALU = mybir.AluOpType
PM = mybir.MatmulPerfMode.DoubleRow

B, S, H = 2, 2048, 2048
NH, KVH, D = 16, 4, 128
NCORES = 8
HPC = NH // KVH  # q heads per core = 4
SB = S // 512  # 4 row-blocks of 512
NB_HC = H // 128  # 16 contraction chunks
SC = S // 128  # 16 k-chunks
EXP_BIAS = -40.0

# fp8 scale plan (powers of two; compensated exactly)
SX = 16.0  # x pre-scale
SWQ = 4096.0  # wq pre-scale (1/sqrt(D) folded into the weights too)
SW = 512.0  # wk/wv/wo pre-scale
OUT_DIV = 8192.0  # host divide: 16 * 512

# w8 per-chunk column layout (CW wide)
CW = 1664
QHI, KHI, QLO, KLO, VHI, VHI2, VLO = 0, 512, 640, 1152, 1280, 1408, 1536
XW = 1024  # x8 per-chunk [hi 512 | lo 512]

LAST_EXEC_TIME_NS = None
LAST_RESULTS = None


def build_graph(reps=1):
    nc = bacc.Bacc(
        "TRN2", target_bir_lowering=False, debug=False, num_devices=NCORES
    )
    xTr8 = nc.dram_tensor("xTr8", [SB * 128, NB_HC * XW], F8, kind="ExternalInput").ap()
    w8d = nc.dram_tensor("w8d", [128, NB_HC * CW], F8, kind="ExternalInput").ap()
    wo8d = nc.dram_tensor("wo8d", [128, HPC * 4096], F8, kind="ExternalInput").ap()
    cosqd = nc.dram_tensor("cosqd", [D, S], BF16, kind="ExternalInput").ap()
    sinqd = nc.dram_tensor("sinqd", [D, S], BF16, kind="ExternalInput").ap()
    coskd = nc.dram_tensor("coskd", [D, S], BF16, kind="ExternalInput").ap()
    sinkd = nc.dram_tensor("sinkd", [D, S], BF16, kind="ExternalInput").ap()
    # out'[feat, s] = 8192 * (attn_out @ wo partial); host transposes/sums
    outp = nc.dram_tensor("outp", [H, S], BF16, kind="ExternalOutput").ap()

    with tile.TileContext(nc) as tc, ExitStack() as ctx:
        const_pool = ctx.enter_context(tc.tile_pool(name="const", bufs=1))
        w8 = const_pool.tile([128, NB_HC * CW], F8)
        wo8 = const_pool.tile([128, HPC * 4096], F8)
        cosq_sb = const_pool.tile([128, S], BF16)
        sinq_sb = const_pool.tile([128, S], BF16)
        cosk_sb = const_pool.tile([128, S], BF16)
        sink_sb = const_pool.tile([128, S], BF16)
        ones_sb = const_pool.tile([128, 128], BF16)  # rowsum lhsT; value 512
        mask_sb = const_pool.tile([128, 128], BF16)  # causal tri mask (col>=p)
        expb_sb = const_pool.tile([128, 1], F32)
        scr_sb = const_pool.tile([128, 1], F32)
        qk_sb = {
            (cg, rb): const_pool.tile([128, 512], BF16, name=f"qk{cg}_{rb}")
            for cg in range(HPC)
            for rb in range(SB)
        }
        kT_sb = const_pool.tile([128, S], BF16)  # roped kT (x8192)
        vall_sb = const_pool.tile([128, S], BF16)  # natural V (x8192)
        outT8 = const_pool.tile([128, HPC * 4096], F8)  # per head: [hi|lo]

        nc.gpsimd.memset(ones_sb[:], SW)  # 512: folds wo-scale into rowsum
        nc.gpsimd.memset(mask_sb[:], 1.0)
        nc.gpsimd.affine_select(
            out=mask_sb[:], in_=mask_sb[:], pattern=[[1, 128]],
            compare_op=ALU.is_ge, fill=0.0, base=0, channel_multiplier=-1,
        )
        nc.gpsimd.memset(expb_sb[:], EXP_BIAS)
        # preheat the ACT Exp table while ACT is idle
        nc.scalar.activation(scr_sb[:], expb_sb[:], AF.Exp, bias=0.0, scale=1.0)

        xt_pool = ctx.enter_context(tc.tile_pool(name="xt", bufs=2))
        rtmp_pool = ctx.enter_context(tc.tile_pool(name="rtmp", bufs=8))
        pt_pool = ctx.enter_context(tc.tile_pool(name="pt", bufs=14))
        s2_pool = ctx.enter_context(tc.tile_pool(name="s2", bufs=6))
        s4_pool = ctx.enter_context(tc.tile_pool(name="s4", bufs=8))
        rr_sb_pool = ctx.enter_context(tc.tile_pool(name="rr_sb", bufs=4))
        ot16_pool = ctx.enter_context(tc.tile_pool(name="ot16", bufs=4))
        osb_pool = ctx.enter_context(tc.tile_pool(name="osb", bufs=6))

        wo3, wo3p, oT3, oT3p = {}, {}, {}, {}
        for ch in range(HPC):
            wo3[ch] = wo8[:, ch * 4096 : (ch + 1) * 4096].rearrange(
                "p (two n) -> p two n", two=2
            )
            oT3[ch] = outT8[:, ch * 4096 : (ch + 1) * 4096].rearrange(
                "p (two n) -> p two n", two=2
            )
        for cp in range(HPC // 2):
            wo3p[cp] = wo8[:, cp * 8192 : (cp + 1) * 8192].rearrange(
                "p (two n) -> p two n", two=2
            )
            oT3p[cp] = outT8[:, cp * 8192 : (cp + 1) * 8192].rearrange(
                "p (two n) -> p two n", two=2
            )

        # ---- o_proj drip FIFO: unit = (qb, f) one [128 feat, 512 row] tile
        pending = []
        ncopy = [0]
        op_pool_ref = [None]

        def emit_op(nmax, split=False, defer_below=0):
            for _ in range(nmax):
                if len(pending) <= defer_below:
                    return
                qb, f = pending.pop(0)
                op_ps = op_pool_ref[0].tile([128, 512], F32, tag="op", name="op_ps")
                for rhalf in range(2):
                    cols = slice(rhalf * 256, (rhalf + 1) * 256)
                    q0 = qb * 512 + rhalf * 256
                    for ch in range(HPC):
                        lhsT = wo8[:, ch * 4096 + f * 128 : ch * 4096 + (f + 1) * 128]
                        lhsT = lhsT.unsqueeze(1).broadcast_to([128, 2, 128])
                        nc.tensor.matmul(
                            op_ps[:, cols],
                            lhsT,
                            oT3[ch][:, :, q0 : q0 + 256],
                            start=(ch == 0),
                            stop=False,
                            perf_mode=PM,
                        )
                    for cp in range(HPC // 2):
                        nc.tensor.matmul(
                            op_ps[:, cols],
                            wo3p[cp][:, :, 2048 + f * 128 : 2048 + (f + 1) * 128],
                            oT3p[cp][:, :, q0 : q0 + 256],
                            start=False,
                            stop=(cp == HPC // 2 - 1),
                            perf_mode=PM,
                        )
                osb = osb_pool.tile([128, 512], BF16, tag="osb", name="osb")
                if split:
                    # tail drain: per-half copies on both engines shorten the
                    # last copy->DMA chain
                    nc.vector.tensor_copy(osb[:, 0:256], op_ps[:, 0:256])
                    nc.scalar.copy(osb[:, 256:512], op_ps[:, 256:512])
                else:
                    if ncopy[0] % 2 == 1:
                        nc.scalar.copy(osb[:], op_ps[:])
                    else:
                        nc.vector.tensor_copy(osb[:], op_ps[:])
                ncopy[0] += 1
                nc.sync.dma_start(
                    outp[f * 128 : (f + 1) * 128, qb * 512 : (qb + 1) * 512], osb[:]
                )

        xts = {}

        def fetch(row0, key, granularity=2):
            t = xt_pool.tile([128, NB_HC * XW], F8, tag="xt")
            step = NB_HC // granularity
            for g in range(granularity):
                sl = slice(g * step * XW, (g + 1) * step * XW)
                nc.sync.dma_start(t[:, sl], xTr8[row0 : row0 + 128, sl])
            xts[key] = t

        def rope(ps, raw, dst, pos, cos_t, sin_t):
            # q' = q*cos + rot(q)*sin; sin table has rotate_half's sign folded
            cs = cos_t[:, pos * 512 : (pos + 1) * 512]
            sn_lo = sin_t[0:64, pos * 512 : (pos + 1) * 512]
            sn_hi = sin_t[64:128, pos * 512 : (pos + 1) * 512]
            t1 = rtmp_pool.tile([128, 512], BF16, tag="rtmp")
            t2 = rtmp_pool.tile([128, 512], BF16, tag="rtmp")
            nc.vector.tensor_mul(t2[0:64, :], ps[64:128, :], sn_lo)
            nc.vector.tensor_mul(t2[64:128, :], ps[0:64, :], sn_hi)
            nc.vector.tensor_mul(t1[:], raw[:], cs)
            nc.vector.tensor_add(dst, t1[:], t2[:])

        def x3_main(xt, c, half):
            return xt[:, c * XW : (c + 1) * XW].rearrange(
                "p (two n) -> p two n", two=2
            )[:, :, half * 256 : (half + 1) * 256]

        def x3_pair(xt, cp, lo, w):
            return xt[:, 2 * cp * XW : (2 * cp + 2) * XW].rearrange(
                "p (two n) -> p two n", two=2
            )[:, :, lo : lo + w]

        def w3_pair(cp, lo, width):
            return w8[:, 2 * cp * CW : (2 * cp + 2) * CW].rearrange(
                "p (two n) -> p two n", two=2
            )[:, :, lo : lo + width]

        def w_dup(c, off, width=128):
            return (
                w8[:, c * CW + off : c * CW + off + width]
                .unsqueeze(1)
                .broadcast_to([128, 2, width])
            )

        # ---------- projection emitters (usable inline or as fill closures)
        def emit_q_pair(pool, xt, rb, cg0):
            qps = {
                cg: pool.tile([128, 512], F32, tag="pj", name=f"q{cg}")
                for cg in (cg0, cg0 + 1)
            }
            for half in range(2):
                cols = slice(half * 256, (half + 1) * 256)
                for c in range(NB_HC):
                    for cg in (cg0, cg0 + 1):
                        nc.tensor.matmul(
                            qps[cg][:, cols],
                            w_dup(c, QHI + cg * 128),
                            x3_main(xt, c, half),
                            start=(c == 0), stop=False, perf_mode=PM,
                        )
                for cp in range(NB_HC // 2):
                    for cg in (cg0, cg0 + 1):
                        nc.tensor.matmul(
                            qps[cg][:, cols],
                            w3_pair(cp, QLO + cg * 128, 128),
                            x3_pair(xt, cp, half * 256, 256),
                            start=False, stop=(cp == NB_HC // 2 - 1),
                            perf_mode=PM,
                        )
            for cg in (cg0, cg0 + 1):
                raw = rtmp_pool.tile([128, 512], BF16, tag="rtmp")
                nc.scalar.copy(raw[:], qps[cg][:])
                rope(qps[cg], raw, qk_sb[(cg, rb)][:], rb, cosq_sb, sinq_sb)

        def k_steps(pool, xt, rb):
            # yields closures: fine-grained emission units for interleaving
            kps = [None]

            def alloc():
                kps[0] = pool.tile([128, 512], F32, tag="kv", name="k")

            yield alloc
            for half in range(2):
                cols = slice(half * 256, (half + 1) * 256)
                for c0 in (0, 4, 8, 12):
                    def main(half=half, cols=cols, c0=c0):
                        for c in range(c0, c0 + 4):
                            nc.tensor.matmul(
                                kps[0][:, cols], w_dup(c, KHI), x3_main(xt, c, half),
                                start=(c == 0), stop=False, perf_mode=PM,
                            )
                    yield main
                for g0 in (0, 4):
                    def corr(half=half, cols=cols, g0=g0):
                        for cp in range(g0, g0 + 4):
                            nc.tensor.matmul(
                                kps[0][:, cols],
                                w3_pair(cp, KLO, 128),
                                x3_pair(xt, cp, half * 256, 256),
                                start=False, stop=(cp == NB_HC // 2 - 1),
                                perf_mode=PM,
                            )
                    yield corr

            def finish():
                kraw = rtmp_pool.tile([128, 512], BF16, tag="rtmp")
                nc.scalar.copy(kraw[:], kps[0][:])
                rope(kps[0], kraw, kT_sb[:, rb * 512 : (rb + 1) * 512], rb,
                     cosk_sb, sink_sb)
            yield finish

        def v_steps(pool, xt, rb):
            vps = [None]

            def alloc():
                vps[0] = pool.tile([128, 512], F32, tag="kv", name="v")

            yield alloc
            for rc in range(4):
                for c0 in (0, 8):
                    def main(rc=rc, c0=c0):
                        vcols = slice(rc * 128, (rc + 1) * 128)
                        for c in range(c0, c0 + 8):
                            lhsT = xt[:, c * XW : (c + 1) * XW].rearrange(
                                "p (two n) -> p two n", two=2
                            )[:, :, rc * 128 : (rc + 1) * 128]
                            nc.tensor.matmul(
                                vps[0][:, vcols],
                                lhsT,
                                w8[:, c * CW + VHI : c * CW + VHI + 256].rearrange(
                                    "p (two n) -> p two n", two=2
                                ),
                                start=(c == 0), stop=False, perf_mode=PM,
                            )
                    yield main

                def corr(rc=rc):
                    vcols = slice(rc * 128, (rc + 1) * 128)
                    for cp in range(NB_HC // 2):
                        nc.tensor.matmul(
                            vps[0][:, vcols],
                            x3_pair(xt, cp, rc * 128, 128),
                            w3_pair(cp, VLO, 128),
                            start=False, stop=(cp == NB_HC // 2 - 1),
                            perf_mode=PM,
                        )
                yield corr

            def drain():
                nc.scalar.copy(vall_sb[:, rb * 512 : (rb + 1) * 512], vps[0][:])
            yield drain

        # ---------- attention pair emitter ----------
        post_q = []  # deferred post-processing closures

        def make_post(h, qb, o_ps_h, r_ps_h):
            def post():
                # halves: o_proj units unblock per 256-col half via subtile
                # deps, so the first drip only waits ~half the split chain.
                rr = rr_sb_pool.tile([128, 512], F32, tag="rr")
                nc.vector.reciprocal(rr[:], r_ps_h[:])
                ot16 = ot16_pool.tile([128, 512], BF16, tag="ot16")
                for w0, w1 in ((0, 256), (256, 512)):
                    cols = slice(w0, w1)
                    nc.vector.tensor_mul(ot16[:, cols], o_ps_h[:, cols], rr[:, cols])
                    hi = outT8[:, h * 4096 + qb * 512 + w0 : h * 4096 + qb * 512 + w1]
                    lo = outT8[:, h * 4096 + 2048 + qb * 512 + w0 :
                               h * 4096 + 2048 + qb * 512 + w1]
                    nc.vector.tensor_copy(hi, ot16[:, cols])
                    nc.gpsimd.tensor_sub(lo, ot16[:, cols], hi)
            return post

        def run_fill(fill, n, floor=0):
            for _ in range(n):
                if len(fill) > floor:
                    fill.pop(0)()

        def attn_pair(qb, pair, ot_pool, rs_pool, st_pool, fill, last=False):
            heads = (2 * pair, 2 * pair + 1)
            o_ps, r_ps = {}, {}
            for h in heads:
                o_ps[h] = ot_pool.tile([128, 512], F32, tag="ot", name=f"ot{h}")
                r_ps[h] = rs_pool.tile([128, 512], F32, tag="rs", name=f"rs{h}")
            nj = 4 * qb + 4
            pd = 3 if last else 5
            pts, s2s, s4s, diag = {}, {}, {}, {}
            for jj in range(nj + pd):
                if jj < 2:
                    # run deferred posts of the previous pair first so the
                    # o_proj units they gate don't stall the PE
                    while post_q:
                        post_q.pop(0)()
                run_fill(fill, 1, floor=6)
                # keep a small reserve of ready units for qb boundaries --
                # freshly enqueued units gate on this qb's posts and would
                # block the in-order PE queue
                emit_op(2 if jj < 1 else 1, defer_below=4)
                if jj < nj:
                    j = jj
                    r = j - 4 * qb
                    qoff = 128 * r if r > 0 else 0
                    for h in heads:
                        s_ps = st_pool.tile([128, 512], F32)
                        nc.tensor.matmul(
                            s_ps[:, qoff:512],
                            kT_sb[:, j * 128 : (j + 1) * 128],
                            qk_sb[(h, qb)][:, qoff:512],
                            start=True,
                            stop=True,
                        )
                        pt = pt_pool.tile([128, 512], BF16)
                        nc.scalar.activation(
                            pt[:, qoff:512],
                            s_ps[:, qoff:512],
                            AF.Exp,
                            bias=expb_sb[:],
                            scale=1.0,
                        )
                        if r >= 0:
                            nc.vector.tensor_mul(
                                pt[:, qoff : qoff + 128],
                                pt[:, qoff : qoff + 128],
                                mask_sb[:],
                            )
                        pts[(h, j)] = (pt, qoff)
                        padd = nc.vector.tensor_add
                        pcopy = nc.vector.tensor_copy
                        if j < 4 * qb:
                            if j % 2 == 1:
                                s2 = s2_pool.tile([128, 512], BF16, tag="s2")
                                padd(s2[:], pts[(h, j - 1)][0][:], pt[:])
                                s2s[(h, j // 2)] = s2
                            if j % 4 == 3:
                                s4 = s4_pool.tile([128, 512], BF16, tag="s4")
                                padd(
                                    s4[:],
                                    s2s.pop((h, j // 2 - 1))[:],
                                    s2s.pop((h, j // 2))[:],
                                )
                                s4s[(h, j // 4)] = s4
                        elif r == 1:
                            pt0 = pts[(h, 4 * qb)][0]
                            sa = s4_pool.tile([128, 512], BF16, tag="s4")
                            pcopy(sa[:, 0:128], pt0[:, 0:128])
                            padd(sa[:, 128:512], pt0[:, 128:512], pt[:, 128:512])
                            diag[(h, 0)] = sa
                        elif r == 3:
                            pt2 = pts[(h, 4 * qb + 2)][0]
                            sb_ = s4_pool.tile([128, 512], BF16, tag="s4")
                            pcopy(sb_[:, 256:384], pt2[:, 256:384])
                            padd(sb_[:, 384:512], pt2[:, 384:512], pt[:, 384:512])
                            diag[(h, 1)] = sb_
                if jj >= pd:
                    j2 = jj - pd
                    for h in heads:
                        pt2, qoff2 = pts.pop((h, j2))
                        if j2 < 4 * qb:
                            if j2 % 4 == 3:
                                s4c = s4s.pop((h, j2 // 4))
                                nc.tensor.matmul(
                                    r_ps[h][:],
                                    ones_sb[:],
                                    s4c[:],
                                    start=(j2 == 3),
                                    stop=False,
                                    skip_group_check=True,
                                )
                        elif j2 == 4 * qb + 1:
                            nc.tensor.matmul(
                                r_ps[h][:],
                                ones_sb[:],
                                diag[(h, 0)][:],
                                start=(qb == 0),
                                stop=False,
                                skip_group_check=True,
                            )
                        elif j2 == 4 * qb + 3:
                            nc.tensor.matmul(
                                r_ps[h][:, 256:512],
                                ones_sb[:],
                                diag[(h, 1)][:, 256:512],
                                start=False,
                                stop=True,
                                skip_group_check=True,
                            )
                        nc.tensor.matmul(
                            o_ps[h][:, qoff2:512],
                            vall_sb[:, j2 * 128 : (j2 + 1) * 128],
                            pt2[:, qoff2:512],
                            start=(j2 == 0),
                            stop=(j2 == nj - 1),
                            skip_group_check=True,
                        )
                run_fill(fill, 1)
                emit_op(1)
            for h in heads:
                post_q.append(make_post(h, qb, o_ps[h], r_ps[h]))

        for _rep in range(reps):
            # ======== phase 1: projections rb0-rb2 + rb3 q (fp8) ===========
            with tc.tile_pool(name="proj_ps", bufs=4, space="PSUM") as proj_pool:
                for rb in range(SB):
                    if rb == 0:
                        # startup: w8 + x(rb0) interleaved on two queues are
                        # the only critical loads; tables/wo8 stream later
                        t = xt_pool.tile([128, NB_HC * XW], F8, tag="xt")
                        xts[0] = t
                        for lo, hi in [(0, 1), (1, 2), (2, 4), (4, 6), (6, 8),
                                       (8, 10), (10, 12), (12, 14), (14, 16)]:
                            nc.sync.dma_start(
                                w8[:, lo * CW : hi * CW], w8d[:, lo * CW : hi * CW]
                            )
                            nc.scalar.dma_start(
                                t[:, lo * XW : hi * XW], xTr8[0:128, lo * XW : hi * XW]
                            )
                    xt = xts.pop(rb)
                    if rb == 0:
                        # rb1's first piece beats the table loads; rb0's rope
                        # tables load as 512-col slices (they're only needed
                        # slice-by-slice, and rb0's q-psum banks are held
                        # until the first rope can run)
                        t = xt_pool.tile([128, NB_HC * XW], F8, tag="xt")
                        xts[1] = t
                        nc.sync.dma_start(t[:, 0 : 4 * XW], xTr8[128:256, 0 : 4 * XW])
                        for tb, dr in ((cosq_sb, cosqd), (sinq_sb, sinqd),
                                       (cosk_sb, coskd), (sink_sb, sinkd)):
                            nc.sync.dma_start(tb[:, 0:512], dr[:, 0:512])
                        for g in range(1, 4):
                            sl = slice(g * 4 * XW, (g + 1) * 4 * XW)
                            nc.sync.dma_start(t[:, sl], xTr8[128:256, sl])
                    elif rb + 1 < SB:
                        fetch((rb + 1) * 128, rb + 1)
                    if rb == 1:
                        for tb, dr in ((cosq_sb, cosqd), (sinq_sb, sinqd),
                                       (cosk_sb, coskd), (sink_sb, sinkd)):
                            nc.scalar.dma_start(tb[:, 512:S], dr[:, 512:S])
                    if rb == 2:
                        nc.scalar.dma_start(wo8[:], wo8d)
                    for cg0 in (0, 2):
                        emit_q_pair(proj_pool, xt, rb, cg0)
                    if rb < SB - 1:
                        for step in k_steps(proj_pool, xt, rb):
                            step()
                        for step in v_steps(proj_pool, xt, rb):
                            step()
                    else:
                        xt_last = xt
            # ======== attention; qb0 interleaved with rb3 k/v ==============
            with (
                tc.tile_pool(name="rs_ps", bufs=2, space="PSUM") as rs_pool,
                tc.tile_pool(name="ot_ps", bufs=2, space="PSUM") as ot_pool,
                tc.tile_pool(name="st_ps", bufs=2, space="PSUM") as st_pool,
            ):
                with tc.tile_pool(name="kv_ps", bufs=2, space="PSUM") as kv_pool:
                    kl = list(k_steps(kv_pool, xt_last, SB - 1))
                    vl = list(v_steps(kv_pool, xt_last, SB - 1))
                    # alloc both psum tiles up front, then interleave the rest
                    kl[0]()
                    vl[0]()
                    fill = kl[1:] + vl[1:]
                    attn_pair(0, 0, ot_pool, rs_pool, st_pool, fill)
                    attn_pair(0, 1, ot_pool, rs_pool, st_pool, fill)
                    # flush pair1's posts first: the leftover k/v fill below
                    # keeps the PE busy while the recip/mul/split chains run
                    while post_q:
                        post_q.pop(0)()
                    run_fill(fill, len(fill))
                    for f in range(16):
                        pending.append((0, f))
                with tc.tile_pool(name="op_ps", bufs=2, space="PSUM") as op_pool:
                    op_pool_ref[0] = op_pool
                    for qb in range(1, SB):
                        for pair in range(2):
                            attn_pair(
                                qb, pair, ot_pool, rs_pool, st_pool, [],
                                last=(qb == SB - 1 and pair == 1),
                            )
                        # end-of-qb: flush pair1's posts, then the reserved
                        # (ready) units cover their recip/mul/split chains
                        while post_q:
                            post_q.pop(0)()
                        emit_op(4)
                        for f in range(16):
                            pending.append((qb, f))
                    emit_op(len(pending) - 8)
            # final drain with more banks once attention psum is closed
            with tc.tile_pool(name="drain_ps", bufs=6, space="PSUM") as drain_pool:
                op_pool_ref[0] = drain_pool
                emit_op(len(pending), split=True)
    nc.compile()
    return nc


_GRAPH = None


def _rope_tables():
    inv_freq = 1.0 / (10000.0 ** (np.arange(0, D, 2, dtype=np.float32) / D))
    t = np.arange(S, dtype=np.float32)
    freqs = np.outer(t, inv_freq)
    emb = np.concatenate([freqs, freqs], axis=-1)  # (S, D)
    cosT = np.ascontiguousarray(np.cos(emb).T.astype(np.float32))
    sinT = np.ascontiguousarray(np.sin(emb).T.astype(np.float32))
    sinadjT = sinT.copy()
    sinadjT[0:64, :] *= -1.0
    return cosT, sinadjT


def _split8(a, f8):
    hi = a.astype(f8)
    lo = (a - hi.astype(np.float32)).astype(f8)
    return hi, lo


def kernel(x, wq, wk, wv, wo):
    global _GRAPH, LAST_EXEC_TIME_NS, LAST_RESULTS
    import ml_dtypes

    f8 = ml_dtypes.float8_e4m3
    bf16 = ml_dtypes.bfloat16
    x = np.asarray(x, dtype=np.float32)
    wq = np.asarray(wq, dtype=np.float32)
    wk = np.asarray(wk, dtype=np.float32)
    wv = np.asarray(wv, dtype=np.float32)
    wo = np.asarray(wo, dtype=np.float32)

    invD = np.float32(1.0 / np.sqrt(D))
    cosT, sinadjT = _rope_tables()
    # q-psum = SX*SWQ*(x@wq/sqrt(D)); roped q must equal true/(SX*SW) so that
    # scores = qk . (SX*SW * k-true) come out exact
    QTS = np.float32(1.0 / (SX * SWQ * SX * SW))
    cosq = (cosT * QTS).astype(bf16)
    sinq = (sinadjT * QTS).astype(bf16)
    cosk = cosT.astype(bf16)
    sink = sinadjT.astype(bf16)

    xg8 = []
    for g in range(B):
        xT = np.ascontiguousarray(x[g].T) * np.float32(SX)  # [H, S]
        xh, xl = _split8(xT, f8)
        xh_r = xh.reshape(NB_HC, 128, SB, 512)
        xl_r = xl.reshape(NB_HC, 128, SB, 512)
        packed = np.stack([xh_r, xl_r], axis=3)  # [hc, p, rb, sel, col]
        xg8.append(
            np.ascontiguousarray(
                packed.transpose(2, 1, 0, 3, 4).reshape(SB * 128, NB_HC * XW)
            )
        )

    w8s, wo8s = [], []
    for kv in range(KVH):
        wq_c = wq[:, kv * HPC * D : (kv + 1) * HPC * D] * (invD * np.float32(SWQ))
        wk_c = wk[:, kv * D : (kv + 1) * D] * np.float32(SW)
        wv_c = wv[:, kv * D : (kv + 1) * D] * np.float32(SW)
        qh, ql = _split8(wq_c, f8)
        kh, kl = _split8(wk_c, f8)
        vh, vl = _split8(wv_c, f8)
        secs = [qh, kh, ql, kl, vh, vh, vl]
        chunk = np.concatenate(
            [s.reshape(NB_HC, 128, -1) for s in secs], axis=2
        )  # [hc, 128, CW]
        w8s.append(
            np.ascontiguousarray(chunk.transpose(1, 0, 2).reshape(128, NB_HC * CW))
        )
        wo_c = wo[kv * HPC * D : (kv + 1) * HPC * D, :] * np.float32(SW)
        oh, ol = _split8(wo_c, f8)
        blk = np.concatenate(
            [oh.reshape(HPC, 128, H), ol.reshape(HPC, 128, H)], axis=2
        )  # [ch, 128, 4096]
        wo8s.append(
            np.ascontiguousarray(blk.transpose(1, 0, 2).reshape(128, HPC * 4096))
        )

    in_maps = []
    for c in range(NCORES):
        g, kv = c // KVH, c % KVH
        in_maps.append(
            {
                "xTr8": xg8[g],
                "w8d": w8s[kv],
                "wo8d": wo8s[kv],
                "cosqd": cosq,
                "sinqd": sinq,
                "coskd": cosk,
                "sinkd": sink,
            }
        )

    if _GRAPH is None:
        _GRAPH = build_graph()

    os.environ["BASS_NEVER_TRACE"] = "1"
    res = None
    for attempt in range(3):
        try:
            res = run_bass_kernel_spmd(
                _GRAPH, in_maps, core_ids=list(range(NCORES))
            )
            break
        except Exception:
            if attempt == 2:
                raise
            time.sleep(5.0)
    LAST_EXEC_TIME_NS = res.exec_time_ns
    LAST_RESULTS = res
    out = np.zeros((B, S, H), dtype=np.float32)
    for c in range(NCORES):
        g = c // KVH
        out[g] += np.asarray(res.results[c]["outp"], dtype=np.float32).T
    out *= np.float32(1.0 / OUT_DIV)
    return out
